# revision 1
# baseline (speedup 1.0000x reference)
"""Trainium2 Bass kernel for nn_ByteEncoder (multi-scale conv stem + per-channel LRU).

Sharding: 8 cores = (batch b in 0..3) x (time-half h in 0..1). Each core runs an
identical SPMD program over raw steps [t0-512, t0+4096) (t0 = h*4096), i.e. a
128-scan-step warmup plus its 1024 output scan steps. The warmup region is
masked to zero for h=0 cores (reference scan starts at state 0) and uses real
left-context for h=1 cores (per-channel decay lambda^128 < 1e-23, so the
truncated history is far below f32 resolution).

The embedding lookup is algebraically fused into the conv stem: for one-hot
inputs, conv_k(embed[x]) == sum_taps (embed @ conv_w[:,:,j])[x[t+off]], so the
stem becomes matmuls of precontracted [256-vocab x 256-ch] tables against
one-hot columns built on-chip (iota + is_equal). This cuts stem FLOPs 4x and
removes the 128MB embedded-activation tensor entirely.

All matmuls run in float32r (full PE rate, ~1.5e-4 matmul rel err measured).
h_multi and h_down bounce through DRAM to keep SBUF within budget.
"""
import numpy as np

import concourse.bass as bass
import concourse.tile as tile
from concourse import mybir, bacc
from concourse.bass_utils import run_bass_kernel_spmd
from concourse.masks import make_identity

P = 128
D = 1024
B = 4
T = 8192
VOCAB = 256
SENTINEL = 512.0  # out-of-range token -> one-hot col is all zero

W_SCAN = 128            # warmup scan steps
S_LOC = 1024 + W_SCAN   # scan steps computed per core (chunk 0 = warmup)
T_LOC = 4 * S_LOC       # raw steps per core (4608)
X_LOC = T_LOC + 8       # x slice incl conv halo (left 4, right 3, +1 pad)
N_TT = T_LOC // 512     # 9 T-tiles in stem
N_CH = S_LOC // 128     # 9 scan chunks

f32 = mybir.dt.float32
f32r = mybir.dt.float32r
AF = mybir.ActivationFunctionType
OP = mybir.AluOpType

# (conv_id, kernel_size, pad); tap offset = j - pad
CONVS = [(1, 0), (2, 1), (4, 2), (8, 4)]
TAPS = []  # (conv_id, j, off)
for ci, (K, pad) in enumerate(CONVS):
    for j in range(K):
        TAPS.append((ci, j, j - pad))
N_TAPS = len(TAPS)  # 15
TAPS_OF_CONV = [[kk for kk, (ci, _, _) in enumerate(TAPS) if ci == c] for c in range(4)]

_CACHE = {}


def _build():
    nc = bacc.Bacc()

    x_d = nc.declare_dram_parameter("x_loc", [X_LOC], mybir.dt.bfloat16, isOutput=False)
    mask_d = nc.declare_dram_parameter("mask", [S_LOC], f32, isOutput=False)
    stem_d = nc.declare_dram_parameter("stem_w", [2, P, N_TAPS, 256], f32r, isOutput=False)
    convb_d = nc.declare_dram_parameter("convb", [P, 8], f32, isOutput=False)
    dw_d = nc.declare_dram_parameter("down_wt", [P, 4, 8, D], f32r, isOutput=False)
    downb_d = nc.declare_dram_parameter("down_b", [D], f32, isOutput=False)
    bw_d = nc.declare_dram_parameter("b_wt", [P, 8, D], f32r, isOutput=False)
    bb2_d = nc.declare_dram_parameter("bb2", [P, 8], f32, isOutput=False)
    cw_d = nc.declare_dram_parameter("c_wt", [P, 8, D], f32r, isOutput=False)
    cb_d = nc.declare_dram_parameter("c_b", [D], f32, isOutput=False)
    slnw_d = nc.declare_dram_parameter("slnw", [D], f32, isOutput=False)
    slnb_d = nc.declare_dram_parameter("slnb", [D], f32, isOutput=False)
    lruw_d = nc.declare_dram_parameter("lruw", [D], f32, isOutput=False)
    lrub_d = nc.declare_dram_parameter("lrub", [D], f32, isOutput=False)
    lam_d = nc.declare_dram_parameter("lam_ct", [P, 8], f32, isOutput=False)

    out_d = nc.declare_dram_parameter("out", [1024, D], f32, isOutput=True)

    # per-tile bounce tensors: a reader then depends only on its own tile's
    # writer, not on the whole phase (Tile tracks DRAM deps per tensor)
    hm_drams = [nc.dram_tensor(f"hm_bounce{t}", [P, 8, 512], f32r)
                for t in range(N_TT)]
    hd_drams = [nc.dram_tensor(f"hd_bounce{c}", [P, D], f32)
                for c in range(N_CH)]

    with tile.TileContext(nc) as tc:
        with tc.tile_pool(name="glob", bufs=1) as glob:
            lam_sb = glob.tile([P, 8], f32, name="lam_sb")
            nc.sync.dma_start(lam_sb[:], lam_d[:])
            convb_sb = glob.tile([P, 8], f32, name="convb_sb")
            nc.sync.dma_start(convb_sb[:], convb_d[:])
            bb2_sb = glob.tile([P, 8], f32, name="bb2_sb")
            nc.sync.dma_start(bb2_sb[:], bb2_d[:])
            eps_sb = glob.tile([P, 1], f32, name="eps_sb")
            nc.vector.memset(eps_sb[:], 1e-5)
            ident = glob.tile([P, P], f32, name="ident")
            make_identity(nc, ident)
            io0 = glob.tile([P, 1], f32, name="io0")
            io1 = glob.tile([P, 1], f32, name="io1")
            nc.gpsimd.iota(io0[:], pattern=[[0, 1]], base=0, channel_multiplier=1,
                           allow_small_or_imprecise_dtypes=True)
            nc.gpsimd.iota(io1[:], pattern=[[0, 1]], base=128, channel_multiplier=1,
                           allow_small_or_imprecise_dtypes=True)

            # p2w opened early: down-conv weights (16.8MB) prefetch during P1
            with tc.tile_pool(name="p2w", bufs=1) as p2w:
              dw_sb = p2w.tile([P, 4, 8, D], f32r, name="dw_sb")
              downb_rep = p2w.tile([P, D], f32, name="downb_rep")
              # hm tiles live here (bufs=2) so the LAST TWO T-tiles' h_multi
              # stay resident across the phase boundary: down-conv S-tiles 7,8
              # then start with no DRAM read while the stem tail still runs
              hmspan = p2w.tile([P, 8, 512], f32r, name="hmspan", bufs=2)
              hm_ts = {}

              # -------------- Phase 1: fused embed+conv stem -> gelu -> hm ----
              with tc.tile_pool(name="p1w", bufs=1) as p1w, \
                 tc.tile_pool(name="p1t", bufs=2) as p1t, \
                 tc.tile_pool(name="ps1", bufs=4, space="PSUM") as ps1:
                stem_sb0 = p1w.tile([P, N_TAPS, 256], f32r, name="stem_sb0")
                stem_sb1 = p1w.tile([P, N_TAPS, 256], f32r, name="stem_sb1")
                stem_sbs = (stem_sb0, stem_sb1)
                nc.sync.dma_start(stem_sb0[:], stem_d[0])

                x_reps = {}

                def issue_xrep(tt):
                    x_rep = p1t.tile([P, 520], mybir.dt.bfloat16, name="x_rep",
                                     bufs=2)
                    nc.sync.dma_start(
                        x_rep[:],
                        x_d[tt * 512: tt * 512 + 520][None, :]
                            .to_broadcast([P, 520]))
                    x_reps[tt] = x_rep

                issue_xrep(0)
                nc.sync.dma_start(stem_sb1[:], stem_d[1])
                issue_xrep(1)
                for tt in range(N_TT):
                    if tt + 2 < N_TT:
                        issue_xrep(tt + 2)
                    if 2 <= tt < 6:
                        # stagger the big down-conv weight load in 4 chunks so
                        # it never starves the small latency-critical DMAs
                        nc.sync.dma_start(dw_sb[:, tt - 2, :, :],
                                          dw_d[:, tt - 2, :, :])
                    if tt == 2:
                        nc.sync.dma_start(downb_rep[:],
                                          downb_d[:][None, :].to_broadcast([P, D]))
                    x_rep = x_reps.pop(tt)
                    oh32 = p1t.tile([P, 2, 520], mybir.dt.bfloat16, name="oh32", bufs=1)
                    nc.vector.tensor_scalar(out=oh32[:, 0, :], in0=x_rep[:],
                                            scalar1=io0[:], scalar2=None,
                                            op0=OP.is_equal)
                    nc.vector.tensor_scalar(out=oh32[:, 1, :], in0=x_rep[:],
                                            scalar1=io1[:], scalar2=None,
                                            op0=OP.is_equal)
                    ohr = p1t.tile([P, 2, 520], f32r, name="ohr")
                    nc.scalar.copy(ohr[:], oh32[:])

                    hm_t = p2w.tile([P, 8, 512], f32r, name="hm_t", tag="hmspan",
                                    bufs=2)
                    hm_ts[tt] = hm_t
                    for cc in range(8):
                        ci, half = cc // 2, cc % 2
                        taps = TAPS_OF_CONV[ci]
                        ps = ps1.tile([P, 512], f32, name="ps", tag="ps")
                        n_mm = len(taps) * 2
                        i = 0
                        for vc in range(2):
                            for kk in taps:
                                off = TAPS[kk][2]
                                nc.tensor.matmul(
                                    ps[:],
                                    stem_sbs[vc][:, kk, half * 128:(half + 1) * 128],
                                    ohr[:, vc, 4 + off: 4 + off + 512],
                                    start=(i == 0), stop=(i == n_mm - 1))
                                i += 1
                        nc.scalar.activation(hm_t[:, cc, :], ps[:], AF.Gelu,
                                             bias=convb_sb[:, cc:cc + 1])
                    if tt < N_TT - 2:
                        nc.sync.dma_start(hm_drams[tt][:], hm_t[:])

              # -------------- Phase 2: strided down-conv (stride 4) ----------
              with tc.tile_pool(name="p2t", bufs=2) as p2t, \
                     tc.tile_pool(name="ps2", bufs=4, space="PSUM") as ps2:
                for s in [N_CH - 2, N_CH - 1] + list(range(N_CH - 2)):
                    if s >= N_CH - 2:
                        hm_sb = hm_ts[s]
                    else:
                        hm_sb = p2t.tile([P, 8, 512], f32r, name="hm_sb")
                        nc.sync.dma_start(hm_sb[:], hm_drams[s][:])
                    for eh in range(2):
                        ps = ps2.tile([P, 512], f32, name="psd", tag="psd")
                        i = 0
                        for dc in range(8):
                            for j in range(4):
                                nc.tensor.matmul(
                                    ps[:],
                                    hm_sb[:, dc, j:512:4],
                                    dw_sb[:, j, dc, eh * 512:(eh + 1) * 512],
                                    start=(i == 0), stop=(i == 31))
                                i += 1
                        hd_t = p2t.tile([P, 512], f32, name="hd_t")
                        nc.vector.tensor_tensor(
                            out=hd_t[:], in0=ps[:],
                            in1=downb_rep[:, eh * 512:(eh + 1) * 512],
                            op=OP.add)
                        nc.sync.dma_start(
                            hd_drams[s][:, eh * 512:(eh + 1) * 512], hd_t[:])

            # ------ Phases 3+4 share one transient pool so the scheduler
            # ------ can interleave c-proj/LN2 with later scan groups -----
            with tc.tile_pool(name="p34w", bufs=1) as p34w, \
                 tc.tile_pool(name="p34t", bufs=2) as p34t, \
                 tc.tile_pool(name="ps3t", bufs=2, space="PSUM") as ps3t, \
                 tc.tile_pool(name="ps3b", bufs=2, space="PSUM") as ps3b, \
                 tc.tile_pool(name="ps4", bufs=2, space="PSUM") as ps4:
                bw_sb = p34w.tile([P, 8, D], f32r, name="bw_sb")
                cw_sb = p34w.tile([P, 8, D], f32r, name="cw_sb")
                z_res = p34w.tile([P, N_CH, D], f32, name="z_res")
                h_all = p34w.tile([P, 8, S_LOC], f32, name="h_all")
                ident_r = p34w.tile([P, P], f32r, name="ident_r")
                nc.scalar.copy(ident_r[:], ident[:])
                mask_rep = p34w.tile([P, 256], f32, name="mask_rep")
                nc.sync.dma_start(mask_rep[:],
                                  mask_d[0:256][None, :].to_broadcast([P, 256]))
                slnw_rep = p34w.tile([P, D], f32, name="slnw_rep")
                cb_rep = p34w.tile([P, D], f32, name="cb_rep")
                slnb_rep = p34w.tile([P, D], f32, name="slnb_rep")
                lruw_rep = p34w.tile([P, D], f32, name="lruw_rep")
                lrub_rep = p34w.tile([P, D], f32, name="lrub_rep")

                # ---- Phase 3: LN -> f32r transpose -> b-proj -> scan ----
                for g0, gn in ((0, 2), (2, 4), (6, 3)):
                    W = gn * 128
                    hsT = p34t.tile([P, 8, 512], f32r, name="hsT", bufs=1)
                    for lc in range(gn):
                        c = g0 + lc
                        hd_c = p34t.tile([P, D], f32, name="hd_c", bufs=2)
                        nc.sync.dma_start(hd_c[:], hd_drams[c][:])
                        if c == 0:
                            nc.sync.dma_start(bw_sb[:], bw_d[:])
                        elif c == 1:
                            nc.sync.dma_start(cw_sb[:], cw_d[:])
                        stats = p34t.tile([P, 2, 6], f32, name="stats", bufs=2)
                        hd_g = hd_c[:].rearrange("p (g f) -> p g f", g=2)
                        nc.vector.bn_stats(out=stats[:, 0, :], in_=hd_g[:, 0, :])
                        nc.vector.bn_stats(out=stats[:, 1, :], in_=hd_g[:, 1, :])
                        mv = p34t.tile([P, 2], f32, name="mv", bufs=2)
                        nc.vector.bn_aggr(out=mv[:], in_=stats[:])
                        rstd = p34t.tile([P, 1], f32, name="rstd", bufs=2)
                        nc.scalar.activation(rstd[:], mv[:, 1:2], AF.Sqrt,
                                             bias=eps_sb[:])
                        nc.vector.reciprocal(rstd[:], rstd[:])
                        nc.vector.tensor_scalar(out=z_res[:, c, :], in0=hd_c[:],
                                                scalar1=mv[:, 0:1],
                                                scalar2=rstd[:],
                                                op0=OP.subtract, op1=OP.mult)
                        zr = p34t.tile([P, D], f32r, name="zr", bufs=1)
                        nc.scalar.copy(zr[:], z_res[:, c, :])
                        for ec in range(8):
                            pst = ps3t.tile([P, P], f32r, name="pst", tag="pst")
                            nc.tensor.transpose(
                                pst[:], zr[:, ec * 128:(ec + 1) * 128],
                                ident_r[:])
                            nc.scalar.copy(
                                hsT[:, ec, lc * 128:(lc + 1) * 128], pst[:])

                    for dc in range(8):
                        psb = ps3b.tile([P, 512], f32, name="psb", tag="psb")
                        for ec in range(8):
                            nc.tensor.matmul(
                                psb[:, :W],
                                bw_sb[:, ec, dc * 128:(dc + 1) * 128],
                                hsT[:, ec, :W],
                                start=(ec == 0), stop=(ec == 7))
                        vals = p34t.tile([P, 512], f32, name="vals", bufs=2)
                        nc.vector.tensor_scalar(out=vals[:, :W], in0=psb[:, :W],
                                                scalar1=bb2_sb[:, dc:dc + 1],
                                                scalar2=None, op0=OP.add)
                        if g0 == 0:
                            # zero the warmup steps (h=0 cores only);
                            # mask is all-ones past chunk 0
                            nc.gpsimd.tensor_tensor(
                                out=vals[:, :W], in0=vals[:, :W],
                                in1=mask_rep[:, :W], op=OP.mult)
                        init = (0.0 if g0 == 0
                                else h_all[:, dc, g0 * 128 - 1: g0 * 128])
                        nc.vector.tensor_tensor_scan(
                            out=h_all[:, dc, g0 * 128: g0 * 128 + W],
                            data0=lam_sb[:, dc:dc + 1].to_broadcast([P, W]),
                            data1=vals[:, :W],
                            initial=init, op0=OP.mult, op1=OP.add)

                # P4 parameter loads issued here, behind the critical P3 DMAs
                nc.sync.dma_start(slnw_rep[:],
                                  slnw_d[:][None, :].to_broadcast([P, D]))
                nc.sync.dma_start(cb_rep[:], cb_d[:][None, :].to_broadcast([P, D]))
                nc.sync.dma_start(slnb_rep[:],
                                  slnb_d[:][None, :].to_broadcast([P, D]))
                # slncb = stem_ln_b + c_b (both added to c_out + h_s)
                nc.vector.tensor_tensor(out=cb_rep[:], in0=cb_rep[:],
                                        in1=slnb_rep[:], op=OP.add)
                nc.sync.dma_start(lruw_rep[:],
                                  lruw_d[:][None, :].to_broadcast([P, D]))
                nc.sync.dma_start(lrub_rep[:],
                                  lrub_d[:][None, :].to_broadcast([P, D]))

                # ---- Phase 4: c-proj + residual + final LN -> out -------
                for c in range(1, N_CH):
                    res = p34t.tile([P, D], f32, name="res", bufs=2)
                    # h_s = z*slnw + slnb; residual = h_s + c_b + c_out
                    nc.gpsimd.tensor_tensor(out=res[:], in0=z_res[:, c, :],
                                            in1=slnw_rep[:], op=OP.mult)
                    nc.gpsimd.tensor_tensor(out=res[:], in0=res[:],
                                            in1=cb_rep[:], op=OP.add)
                    for eh in range(2):
                        psc = ps4.tile([P, 512], f32, name="psc", tag="psc")
                        for dc in range(8):
                            har = p34t.tile([P, P], f32r, name="har", bufs=2)
                            nc.scalar.copy(har[:],
                                           h_all[:, dc, c * 128:(c + 1) * 128])
                            nc.tensor.matmul(
                                psc[:],
                                har[:],
                                cw_sb[:, dc, eh * 512:(eh + 1) * 512],
                                start=(dc == 0), stop=(dc == 7))
                        nc.vector.tensor_tensor(
                            out=res[:, eh * 512:(eh + 1) * 512],
                            in0=psc[:],
                            in1=res[:, eh * 512:(eh + 1) * 512],
                            op=OP.add)

                    stats2 = p34t.tile([P, 2, 6], f32, name="stats2", bufs=2)
                    res_g = res[:].rearrange("p (g f) -> p g f", g=2)
                    nc.vector.bn_stats(out=stats2[:, 0, :], in_=res_g[:, 0, :])
                    nc.vector.bn_stats(out=stats2[:, 1, :], in_=res_g[:, 1, :])
                    mv2 = p34t.tile([P, 2], f32, name="mv2", bufs=2)
                    nc.vector.bn_aggr(out=mv2[:], in_=stats2[:])
                    rstd2 = p34t.tile([P, 1], f32, name="rstd2", bufs=2)
                    nc.scalar.activation(rstd2[:], mv2[:, 1:2], AF.Sqrt,
                                         bias=eps_sb[:])
                    nc.vector.reciprocal(rstd2[:], rstd2[:])
                    o_t = p34t.tile([P, D], f32, name="o_t", bufs=2)
                    nc.vector.tensor_scalar(out=o_t[:], in0=res[:],
                                            scalar1=mv2[:, 0:1],
                                            scalar2=rstd2[:],
                                            op0=OP.subtract, op1=OP.mult)
                    nc.gpsimd.tensor_tensor(out=o_t[:], in0=o_t[:],
                                            in1=lruw_rep[:], op=OP.mult)
                    nc.gpsimd.tensor_tensor(out=o_t[:], in0=o_t[:],
                                            in1=lrub_rep[:], op=OP.add)
                    nc.sync.dma_start(out_d[(c - 1) * 128: c * 128, :], o_t[:])

    nc.finalize()
    return nc


def _prep_host(inputs):
    f = np.float32
    embed = np.asarray(inputs["embed"], f)
    conv_ws = [np.asarray(inputs[k], f) for k in
               ("conv1_w", "conv2_w", "conv4_w", "conv8_w")]
    conv_bs = [np.asarray(inputs[k], f) for k in
               ("conv1_b", "conv2_b", "conv4_b", "conv8_b")]
    down_w = np.asarray(inputs["down_w"], f)
    log_lam = np.asarray(inputs["log_lambda_raw"], f)
    lam = (1.0 / (1.0 + np.exp(-log_lam.astype(np.float64)))).astype(f)
    b_w = np.asarray(inputs["b_w"], f)
    c_w = np.asarray(inputs["c_w"], f)

    stem_w = np.empty((2, P, N_TAPS, 256), f)
    for kk, (ci, j, _off) in enumerate(TAPS):
        fused = embed @ conv_ws[ci][:, :, j].T        # [256v, 256c]
        stem_w[:, :, kk, :] = fused.reshape(2, P, 256)
    convb = np.concatenate(conv_bs).reshape(8, P).T.copy()      # [p, cc]

    down_wt = (down_w.transpose(1, 2, 0)                        # [d, j, e]
               .reshape(8, P, 4, D).transpose(1, 2, 0, 3).copy())  # [p, j, dc, e]
    one_m = (1.0 - lam)
    slnw = np.asarray(inputs["stem_ln_w"], f)
    slnb = np.asarray(inputs["stem_ln_b"], f)
    # values[d,t] = sum_e [(1-lam_d) b_w[d,e] slnw[e]] z^T[e,t]
    #              + (1-lam_d)(b_w[d,:] @ slnb + b_b[d])
    b_wt = ((b_w.T * one_m[None, :] * slnw[:, None])            # [e, d]
            .reshape(8, P, D).transpose(1, 0, 2).copy())        # [p, ec, d]
    bb2 = (one_m * (b_w @ slnb + np.asarray(inputs["b_b"], f))
           ).reshape(8, P).T.copy()
    c_wt = c_w.T.reshape(8, P, D).transpose(1, 0, 2).copy()     # [p, dc, e]
    lam_ct = lam.reshape(8, P).T.copy()

    shared = dict(
        stem_w=stem_w, convb=convb, down_wt=down_wt,
        down_b=np.asarray(inputs["down_b"], f),
        b_wt=b_wt, bb2=bb2, c_wt=c_wt,
        c_b=np.asarray(inputs["c_b"], f),
        slnw=np.asarray(inputs["stem_ln_w"], f),
        slnb=np.asarray(inputs["stem_ln_b"], f),
        lruw=np.asarray(inputs["lru_ln_w"], f),
        lrub=np.asarray(inputs["lru_ln_b"], f),
        lam_ct=lam_ct,
    )

    x = np.asarray(inputs["x"]).astype(np.int64)
    in_maps = []
    for core in range(8):
        b, h = core // 2, core % 2
        t0 = h * 4096
        idx = t0 - 516 + np.arange(X_LOC)
        valid = (idx >= 0) & (idx < T)
        import ml_dtypes
        x_loc = np.full((X_LOC,), SENTINEL, ml_dtypes.bfloat16)
        x_loc[valid] = x[b, idx[valid]].astype(ml_dtypes.bfloat16)
        mask = np.ones((S_LOC,), f)
        if h == 0:
            mask[:W_SCAN] = 0.0
        m = dict(shared)
        m["x_loc"] = x_loc
        m["mask"] = mask
        in_maps.append(m)
    return in_maps


def kernel(**inputs) -> np.ndarray:
    if "nc" not in _CACHE:
        _CACHE["nc"] = _build()
    nc = _CACHE["nc"]
    in_maps = _prep_host(inputs)
    res = run_bass_kernel_spmd(nc, in_maps, list(range(8)))
    out = np.empty((B, 2048, D), np.float32)
    for core in range(8):
        b, h = core // 2, core % 2
        out[b, h * 1024:(h + 1) * 1024, :] = res.results[core]["out"]
    return out



# revision 5
# speedup vs baseline: 1.5018x; 1.5018x over previous
"""Trainium2 Bass kernel for nn_ByteEncoder (multi-scale conv stem + per-channel LRU).

Sharding: 8 cores = (batch b in 0..3) x (time-half h in 0..1). Each core runs an
identical SPMD program over raw steps [t0-512, t0+4096) (t0 = h*4096), i.e. a
128-scan-step warmup plus its 1024 output scan steps. The warmup region is
masked to zero for h=0 cores (reference scan starts at state 0) and uses real
left-context for h=1 cores (per-channel decay lambda^128 < 1e-23, far below f32
resolution).

The embedding lookup is algebraically fused into the conv stem: for one-hot
inputs, conv_k(embed[x]) == sum_taps (embed @ conv_w[:,:,j])[x[t+off]], so the
stem becomes matmuls of precontracted [256-vocab x 256-ch] tables against
one-hot columns built on-chip (iota + is_equal).

All matmuls run in bf16 (full PE rate, fast weight load path, half the SBUF and
HBM traffic of fp32). Everything stays in SBUF: the stem and the strided
down-conv are fused per 512-step tile, and phase 3/4 (LN -> b-proj -> scan ->
c-proj -> LN) is interleaved with later stem tiles so the tensor engine never
idles. Per-channel bias adds are folded into 1-row matmuls (down-conv, masked
b-proj bias) or per-partition scalar-engine bias (b-proj groups 1/2).
"""
import numpy as np

import concourse.bass as bass
import concourse.tile as tile
from concourse import mybir, bacc
from concourse.bass_utils import run_bass_kernel_spmd
from concourse.masks import make_identity

P = 128
D = 1024
B = 4
T = 8192
VOCAB = 256
SENTINEL = 512.0  # out-of-range token -> one-hot col is all zero

W_SCAN = 128            # warmup scan steps
S_LOC = 1024 + W_SCAN   # scan steps computed per core (chunk 0 = warmup)
T_LOC = 4 * S_LOC       # raw steps per core (4608)
X_LOC = T_LOC + 8       # x slice incl conv halo (left 4, right 3, +1 pad)
N_TT = T_LOC // 512     # 9 T-tiles in stem
N_CH = S_LOC // 128     # 9 scan chunks
GROUPS = [(0, 3), (3, 3), (6, 3)]  # (first chunk, n chunks) per scan group

f32 = mybir.dt.float32
bf16 = mybir.dt.bfloat16
AF = mybir.ActivationFunctionType
OP = mybir.AluOpType

# (conv_id, kernel_size, pad); tap offset = j - pad
CONVS = [(1, 0), (2, 1), (4, 2), (8, 4)]
TAPS = []  # (conv_id, j, off)
for ci, (K, pad) in enumerate(CONVS):
    for j in range(K):
        TAPS.append((ci, j, j - pad))
N_TAPS = len(TAPS)  # 15
TAPS_OF_CONV = [[kk for kk, (ci, _, _) in enumerate(TAPS) if ci == c] for c in range(4)]

_CACHE = {}


def _build():
    nc = bacc.Bacc()

    x_d = nc.declare_dram_parameter("x_loc", [X_LOC], bf16, isOutput=False)
    maskc_d = nc.declare_dram_parameter("mask_col", [P, 1], f32, isOutput=False)
    maskr_d = nc.declare_dram_parameter("mask_row", [384], bf16, isOutput=False)
    stem_d = nc.declare_dram_parameter("stem_w", [2, P, N_TAPS, 256], bf16, isOutput=False)
    convb_d = nc.declare_dram_parameter("convb", [P, 8], f32, isOutput=False)
    dw_d = nc.declare_dram_parameter("down_wt", [P, 4, 8, D], bf16, isOutput=False)
    downbr_d = nc.declare_dram_parameter("downb_row", [D], bf16, isOutput=False)
    bw_d = nc.declare_dram_parameter("b_wt", [P, 8, D], bf16, isOutput=False)
    bb2_d = nc.declare_dram_parameter("bb2", [P, 8], f32, isOutput=False)
    bb2r_d = nc.declare_dram_parameter("bb2_row", [D], bf16, isOutput=False)
    cw_d = nc.declare_dram_parameter("c_wt", [P, 8, D], bf16, isOutput=False)
    slnw_d = nc.declare_dram_parameter("slnw_v", [D], bf16, isOutput=False)
    ccb_d = nc.declare_dram_parameter("ccb_v", [D], bf16, isOutput=False)
    lruw_d = nc.declare_dram_parameter("lruw_v", [D], bf16, isOutput=False)
    lrub_d = nc.declare_dram_parameter("lrub_v", [D], bf16, isOutput=False)
    lam_d = nc.declare_dram_parameter("lam_ct", [P, 8], f32, isOutput=False)

    out_d = nc.declare_dram_parameter("out", [1024, D], f32, isOutput=True)

    with tile.TileContext(nc) as tc:
        with tc.tile_pool(name="glob", bufs=1) as glob, \
             tc.tile_pool(name="pw", bufs=1) as pw, \
             tc.tile_pool(name="p12t", bufs=2) as p12t, \
             tc.tile_pool(name="p34t", bufs=2) as p34t, \
             tc.tile_pool(name="ps_stem", bufs=2, space="PSUM") as ps_stem, \
             tc.tile_pool(name="ps_down", bufs=2, space="PSUM") as ps_down, \
             tc.tile_pool(name="ps_tr", bufs=1, space="PSUM") as ps_tr, \
             tc.tile_pool(name="ps_bp", bufs=2, space="PSUM") as ps_bp, \
             tc.tile_pool(name="ps_cp", bufs=1, space="PSUM") as ps_cp:

            # ---------------- small global state -------------------------
            lam_sb = glob.tile([P, 8], f32, name="lam_sb")
            nc.sync.dma_start(lam_sb[:], lam_d[:])
            convb_sb = glob.tile([P, 8], f32, name="convb_sb")
            nc.sync.dma_start(convb_sb[:], convb_d[:])
            bb2_sb = glob.tile([P, 8], f32, name="bb2_sb")
            nc.sync.dma_start(bb2_sb[:], bb2_d[:])
            mask_col = glob.tile([P, 1], f32, name="mask_col")
            nc.sync.dma_start(mask_col[:], maskc_d[:])
            mask_row = glob.tile([1, 384], bf16, name="mask_row")
            nc.sync.dma_start(mask_row[:], maskr_d[:][None, :])
            bb2_row = glob.tile([1, D], bf16, name="bb2_row")
            nc.sync.dma_start(bb2_row[:], bb2r_d[:][None, :])
            downb_row = glob.tile([1, D], bf16, name="downb_row")
            nc.sync.dma_start(downb_row[:], downbr_d[:][None, :])
            eps_sb = glob.tile([P, 1], f32, name="eps_sb")
            nc.vector.memset(eps_sb[:], 1e-5)
            ones_col = glob.tile([1, P], bf16, name="ones_col")
            nc.vector.memset(ones_col[:], 1.0)
            ident = glob.tile([P, P], bf16, name="ident")
            make_identity(nc, ident)
            io0 = glob.tile([P, 1], f32, name="io0")
            io1 = glob.tile([P, 1], f32, name="io1")
            nc.gpsimd.iota(io0[:], pattern=[[0, 1]], base=0, channel_multiplier=1,
                           allow_small_or_imprecise_dtypes=True)
            nc.gpsimd.iota(io1[:], pattern=[[0, 1]], base=128, channel_multiplier=1,
                           allow_small_or_imprecise_dtypes=True)

            # persistent activations
            z_bf = glob.tile([P, N_CH, D], bf16, name="z_bf")

            # big weights
            stem_sb0 = pw.tile([P, N_TAPS, 256], bf16, name="stem_sb0")
            stem_sb1 = pw.tile([P, N_TAPS, 256], bf16, name="stem_sb1")
            stem_sbs = (stem_sb0, stem_sb1)
            dw_sb = pw.tile([P, 4, 8, D], bf16, name="dw_sb")
            bw_sb = pw.tile([P, 8, D], bf16, name="bw_sb")
            cw_sb = pw.tile([P, 8, D], bf16, name="cw_sb")
            slnw_rep = pw.tile([P, D], bf16, name="slnw_rep")
            ccb_rep = pw.tile([P, D], bf16, name="ccb_rep")
            lruw_rep = pw.tile([P, D], bf16, name="lruw_rep")
            lrub_rep = pw.tile([P, D], bf16, name="lrub_rep")

            # first weight loads (stem tables + x windows before anything else)
            nc.sync.dma_start(stem_sb0[:], stem_d[0])

            x_reps = {}

            def issue_xrep(tt):
                x_rep = p12t.tile([P, 520], bf16, name="x_rep", bufs=2)
                nc.sync.dma_start(
                    x_rep[:],
                    x_d[tt * 512: tt * 512 + 520][None, :].to_broadcast([P, 520]))
                x_reps[tt] = x_rep

            issue_xrep(0)
            nc.sync.dma_start(stem_sb1[:], stem_d[1])
            issue_xrep(1)

            hm_ts = {}

            def stem(tt):
                x_rep = x_reps.pop(tt)
                oh = p12t.tile([P, 2, 520], bf16, name="oh", bufs=2)
                nc.vector.tensor_scalar(out=oh[:, 0, :], in0=x_rep[:],
                                        scalar1=io0[:], scalar2=None,
                                        op0=OP.is_equal)
                nc.vector.tensor_scalar(out=oh[:, 1, :], in0=x_rep[:],
                                        scalar1=io1[:], scalar2=None,
                                        op0=OP.is_equal)
                hm_t = p12t.tile([P, 8, 512], bf16, name="hm_t", bufs=2)
                hm_ts[tt] = hm_t
                for cc in range(8):
                    ci, half = cc // 2, cc % 2
                    taps = TAPS_OF_CONV[ci]
                    ps = ps_stem.tile([P, 512], f32, name="pss", tag="pss")
                    n_mm = len(taps) * 2
                    i = 0
                    for vc in range(2):
                        for kk in taps:
                            off = TAPS[kk][2]
                            nc.tensor.matmul(
                                ps[:],
                                stem_sbs[vc][:, kk, half * 128:(half + 1) * 128],
                                oh[:, vc, 4 + off: 4 + off + 512],
                                start=(i == 0), stop=(i == n_mm - 1))
                            i += 1
                    nc.scalar.activation(hm_t[:, cc, :], ps[:], AF.Gelu,
                                         bias=convb_sb[:, cc:cc + 1])

            hd_ts = {}

            def down(c):
                hm_sb = hm_ts.pop(c)
                hd_t = p34t.tile([P, D], bf16, name="hd_t", tag="hd", bufs=2)
                hd_ts[c] = hd_t
                for eh in range(2):
                    ps = ps_down.tile([P, 512], f32, name="psd", tag="psd")
                    # bias row via 1-row matmul: psd = ones^T . downb_row + ...
                    nc.tensor.matmul(
                        ps[:], ones_col[:, :], downbr_slice(eh),
                        start=True, stop=False)
                    i = 0
                    for dc in range(8):
                        for j in range(4):
                            nc.tensor.matmul(
                                ps[:],
                                hm_sb[:, dc, j:512:4],
                                dw_sb[:, j, dc, eh * 512:(eh + 1) * 512],
                                start=False, stop=(i == 31))
                            i += 1
                    nc.scalar.copy(hd_t[:, eh * 512:(eh + 1) * 512], ps[:])

            def downbr_slice(eh):
                return downb_row[:, eh * 512:(eh + 1) * 512]

            def lnt(c):
                """LN stats + z + transpose for chunk c."""
                g, lc = next((gi, c - g0) for gi, (g0, gn) in enumerate(GROUPS)
                             if g0 <= c < g0 + gn)
                hd_t = hd_ts.pop(c)
                stats = p34t.tile([P, 2, 6], f32, name="stats", bufs=2)
                hd_g = hd_t[:].rearrange("p (g f) -> p g f", g=2)
                nc.vector.bn_stats(out=stats[:, 0, :], in_=hd_g[:, 0, :])
                nc.vector.bn_stats(out=stats[:, 1, :], in_=hd_g[:, 1, :])
                mv = p34t.tile([P, 2], f32, name="mv", bufs=2)
                nc.vector.bn_aggr(out=mv[:], in_=stats[:])
                rstd = p34t.tile([P, 1], f32, name="rstd", bufs=2)
                nc.scalar.activation(rstd[:], mv[:, 1:2], AF.Sqrt, bias=eps_sb[:])
                nc.vector.reciprocal(rstd[:], rstd[:])
                if c == 0:
                    # zero warmup z on h=0 cores (mask) by zeroing rstd
                    nc.vector.tensor_tensor(out=rstd[:], in0=rstd[:],
                                            in1=mask_col[:], op=OP.mult)
                nc.vector.tensor_scalar(out=z_bf[:, c, :], in0=hd_t[:],
                                        scalar1=mv[:, 0:1], scalar2=rstd[:],
                                        op0=OP.subtract, op1=OP.mult)
                pst = ps_tr.tile([P, 8, P], bf16, name="pst", tag="pst")
                for ec in range(8):
                    nc.tensor.transpose(
                        pst[:, ec, :], z_bf[:, c, ec * 128:(ec + 1) * 128],
                        ident[:])
                hsT_g = hsT_tiles[g]
                nc.scalar.copy(hsT_g[:, :, lc * 128:(lc + 1) * 128], pst[:])

            hsT_tiles = {}
            h_tiles = {}

            def open_group(g):
                hsT_tiles[g] = p34t.tile([P, 8, 384], bf16, name="hsT",
                                         tag="hsT", bufs=2)

            def bproj_scan(g):
                g0, gn = GROUPS[g]
                W = gn * 128
                hsT_g = hsT_tiles[g]
                h_g = p34t.tile([P, 8, 384], bf16, name="h_g", tag="h_g", bufs=2)
                h_tiles[g] = h_g
                for dc in range(8):
                    psb = ps_bp.tile([P, 384], f32, name="psb", tag="psb")
                    if g == 0:
                        # masked per-channel bias via 1-row matmul
                        nc.tensor.matmul(psb[:, :W],
                                         bb2_row[:, dc * 128:(dc + 1) * 128],
                                         mask_row[:, :W],
                                         start=True, stop=False)
                    for ec in range(8):
                        nc.tensor.matmul(
                            psb[:, :W],
                            bw_sb[:, ec, dc * 128:(dc + 1) * 128],
                            hsT_g[:, ec, :W],
                            start=(g != 0 and ec == 0), stop=(ec == 7))
                    vals = p34t.tile([P, 384], bf16, name="vals", bufs=2)
                    if g == 0:
                        nc.scalar.copy(vals[:, :W], psb[:, :W])
                    else:
                        nc.scalar.activation(vals[:, :W], psb[:, :W],
                                             AF.Identity,
                                             bias=bb2_sb[:, dc:dc + 1])
                    init = (0.0 if g == 0
                            else h_tiles[g - 1][:, dc, 383:384])
                    nc.vector.tensor_tensor_scan(
                        out=h_g[:, dc, :W],
                        data0=lam_sb[:, dc:dc + 1].to_broadcast([P, W]),
                        data1=vals[:, :W],
                        initial=init, op0=OP.mult, op1=OP.add)

            def p4(c):
                """c-proj + residual + final LN -> out rows (c-1)*128.."""
                g, lc = next((gi, c - g0) for gi, (g0, gn) in enumerate(GROUPS)
                             if g0 <= c < g0 + gn)
                h_g = h_tiles[g]
                res_b = p34t.tile([P, D], bf16, name="res_b", bufs=2)
                nc.gpsimd.tensor_tensor(out=res_b[:], in0=z_bf[:, c, :],
                                        in1=slnw_rep[:], op=OP.mult)
                nc.gpsimd.tensor_tensor(out=res_b[:], in0=res_b[:],
                                        in1=ccb_rep[:], op=OP.add)
                res_f = p34t.tile([P, D], f32, name="res_f", bufs=2)
                for eh in range(2):
                    psc = ps_cp.tile([P, 512], f32, name="psc", tag="psc")
                    for dc in range(8):
                        nc.tensor.matmul(
                            psc[:],
                            h_g[:, dc, lc * 128:(lc + 1) * 128],
                            cw_sb[:, dc, eh * 512:(eh + 1) * 512],
                            start=(dc == 0), stop=(dc == 7))
                    nc.vector.tensor_tensor(
                        out=res_f[:, eh * 512:(eh + 1) * 512], in0=psc[:],
                        in1=res_b[:, eh * 512:(eh + 1) * 512], op=OP.add)
                stats2 = p34t.tile([P, 2, 6], f32, name="stats2", bufs=2)
                res_g = res_f[:].rearrange("p (g f) -> p g f", g=2)
                nc.vector.bn_stats(out=stats2[:, 0, :], in_=res_g[:, 0, :])
                nc.vector.bn_stats(out=stats2[:, 1, :], in_=res_g[:, 1, :])
                mv2 = p34t.tile([P, 2], f32, name="mv2", bufs=2)
                nc.vector.bn_aggr(out=mv2[:], in_=stats2[:])
                rstd2 = p34t.tile([P, 1], f32, name="rstd2", bufs=2)
                nc.scalar.activation(rstd2[:], mv2[:, 1:2], AF.Sqrt,
                                     bias=eps_sb[:])
                nc.vector.reciprocal(rstd2[:], rstd2[:])
                nc.vector.tensor_scalar(out=res_f[:], in0=res_f[:],
                                        scalar1=mv2[:, 0:1], scalar2=rstd2[:],
                                        op0=OP.subtract, op1=OP.mult)
                nc.vector.tensor_tensor(out=res_f[:], in0=res_f[:],
                                        in1=lruw_rep[:], op=OP.mult)
                nc.vector.tensor_tensor(out=res_f[:], in0=res_f[:],
                                        in1=lrub_rep[:], op=OP.add)
                nc.sync.dma_start(out_d[(c - 1) * 128: c * 128, :], res_f[:])

            # ---------------- software-pipelined emission ----------------
            open_group(0)
            stem(0)
            issue_xrep(2)
            nc.sync.dma_start(dw_sb[:, 0, :, :], dw_d[:, 0, :, :])
            nc.sync.dma_start(dw_sb[:, 1, :, :], dw_d[:, 1, :, :])
            stem(1)
            issue_xrep(3)
            nc.sync.dma_start(dw_sb[:, 2, :, :], dw_d[:, 2, :, :])
            nc.sync.dma_start(dw_sb[:, 3, :, :], dw_d[:, 3, :, :])
            stem(2)
            issue_xrep(4)
            down(0)
            nc.sync.dma_start(bw_sb[:], bw_d[:])
            down(1)
            stem(3)
            issue_xrep(5)
            down(2)
            lnt(0)
            nc.sync.dma_start(slnw_rep[:], slnw_d[:][None, :].to_broadcast([P, D]))
            nc.sync.dma_start(ccb_rep[:], ccb_d[:][None, :].to_broadcast([P, D]))
            stem(4)
            issue_xrep(6)
            down(3)
            lnt(1)
            nc.sync.dma_start(cw_sb[:], cw_d[:])
            stem(5)
            issue_xrep(7)
            down(4)
            lnt(2)
            bproj_scan(0)
            open_group(1)
            nc.sync.dma_start(lruw_rep[:], lruw_d[:][None, :].to_broadcast([P, D]))
            nc.sync.dma_start(lrub_rep[:], lrub_d[:][None, :].to_broadcast([P, D]))
            stem(6)
            issue_xrep(8)
            down(5)
            lnt(3)
            p4(1)
            stem(7)
            down(6)
            lnt(4)
            p4(2)
            stem(8)
            down(7)
            lnt(5)
            bproj_scan(1)
            open_group(2)
            down(8)
            lnt(6)
            p4(3)
            p4(4)
            lnt(7)
            p4(5)
            lnt(8)
            bproj_scan(2)
            p4(6)
            p4(7)
            p4(8)

    nc.finalize()
    return nc


def _prep_host(inputs):
    import ml_dtypes
    f = np.float32
    bf = ml_dtypes.bfloat16
    embed = np.asarray(inputs["embed"], f)
    conv_ws = [np.asarray(inputs[k], f) for k in
               ("conv1_w", "conv2_w", "conv4_w", "conv8_w")]
    conv_bs = [np.asarray(inputs[k], f) for k in
               ("conv1_b", "conv2_b", "conv4_b", "conv8_b")]
    down_w = np.asarray(inputs["down_w"], f)
    log_lam = np.asarray(inputs["log_lambda_raw"], f)
    lam = (1.0 / (1.0 + np.exp(-log_lam.astype(np.float64)))).astype(f)
    b_w = np.asarray(inputs["b_w"], f)
    c_w = np.asarray(inputs["c_w"], f)

    stem_w = np.empty((2, P, N_TAPS, 256), f)
    for kk, (ci, j, _off) in enumerate(TAPS):
        fused = embed @ conv_ws[ci][:, :, j].T        # [256v, 256c]
        stem_w[:, :, kk, :] = fused.reshape(2, P, 256)
    convb = np.concatenate(conv_bs).reshape(8, P).T.copy()      # [p, cc]

    down_wt = (down_w.transpose(1, 2, 0)                        # [d, j, e]
               .reshape(8, P, 4, D).transpose(1, 2, 0, 3).copy())  # [p, j, dc, e]
    one_m = (1.0 - lam)
    slnw = np.asarray(inputs["stem_ln_w"], f)
    slnb = np.asarray(inputs["stem_ln_b"], f)
    # values[d,t] = sum_e [(1-lam_d) b_w[d,e] slnw[e]] z^T[e,t]
    #              + (1-lam_d)(b_w[d,:] @ slnb + b_b[d])
    b_wt = ((b_w.T * one_m[None, :] * slnw[:, None])            # [e, d]
            .reshape(8, P, D).transpose(1, 0, 2).copy())        # [p, ec, d]
    bb2 = (one_m * (b_w @ slnb + np.asarray(inputs["b_b"], f))
           ).reshape(8, P).T.copy()                             # [p, dc]
    bb2_row = (one_m * (b_w @ slnb + np.asarray(inputs["b_b"], f)))  # [d]
    c_wt = c_w.T.reshape(8, P, D).transpose(1, 0, 2).copy()     # [p, dc, e]
    lam_ct = lam.reshape(8, P).T.copy()
    ccb = slnb + np.asarray(inputs["c_b"], f)

    shared = dict(
        stem_w=stem_w.astype(bf), convb=convb,
        down_wt=down_wt.astype(bf),
        downb_row=np.asarray(inputs["down_b"], f).astype(bf),
        b_wt=b_wt.astype(bf), bb2=bb2, bb2_row=bb2_row.astype(bf),
        c_wt=c_wt.astype(bf),
        slnw_v=slnw.astype(bf), ccb_v=ccb.astype(bf),
        lruw_v=np.asarray(inputs["lru_ln_w"], f).astype(bf),
        lrub_v=np.asarray(inputs["lru_ln_b"], f).astype(bf),
        lam_ct=lam_ct,
    )

    x = np.asarray(inputs["x"]).astype(np.int64)
    in_maps = []
    for core in range(8):
        b, h = core // 2, core % 2
        t0 = h * 4096
        idx = t0 - 516 + np.arange(X_LOC)
        valid = (idx >= 0) & (idx < T)
        x_loc = np.full((X_LOC,), SENTINEL, bf)
        x_loc[valid] = x[b, idx[valid]].astype(bf)
        mask = np.ones((S_LOC,), f)
        if h == 0:
            mask[:W_SCAN] = 0.0
        m = dict(shared)
        m["x_loc"] = x_loc
        m["mask_col"] = mask[:P].reshape(P, 1).copy()
        m["mask_row"] = mask[:384].astype(bf)
        in_maps.append(m)
    return in_maps


def kernel(**inputs) -> np.ndarray:
    if "nc" not in _CACHE:
        _CACHE["nc"] = _build()
    nc = _CACHE["nc"]
    in_maps = _prep_host(inputs)
    res = run_bass_kernel_spmd(nc, in_maps, list(range(8)))
    out = np.empty((B, 2048, D), np.float32)
    for core in range(8):
        b, h = core // 2, core % 2
        out[b, h * 1024:(h + 1) * 1024, :] = res.results[core]["out"]
    return out


# revision 24
# speedup vs baseline: 1.5425x; 1.0271x over previous
"""Trainium2 Bass kernel for nn_ByteEncoder (multi-scale conv stem + per-channel LRU).

Sharding: 8 cores = (batch b in 0..3) x (time-half h in 0..1). Each core runs an
identical SPMD program over raw steps [t0-512, t0+4096) (t0 = h*4096), i.e. a
128-scan-step warmup plus its 1024 output scan steps. The warmup region is
masked to zero for h=0 cores (reference scan starts at state 0) and uses real
left-context for h=1 cores (per-channel decay lambda^128 < 1e-23, far below f32
resolution).

The embedding lookup is algebraically fused into the conv stem: for one-hot
inputs, conv_k(embed[x]) == sum_taps (embed @ conv_w[:,:,j])[x[t+off]], so the
stem becomes matmuls of precontracted [256-vocab x 256-ch] tables against
one-hot columns built on-chip (iota + is_equal).

All matmuls run in bf16 (full PE rate, fast weight load path, half the SBUF and
HBM traffic of fp32). Everything stays in SBUF: the stem and the strided
down-conv are fused per 512-step tile, and phase 3/4 (LN -> b-proj -> scan ->
c-proj -> LN) is interleaved with later stem tiles so the tensor engine never
idles. Per-channel bias adds are folded into 1-row matmuls (down-conv, masked
b-proj bias) or per-partition scalar-engine bias (b-proj groups 1/2).
"""
import numpy as np

import concourse.bass as bass
import concourse.tile as tile
from concourse import mybir, bacc
from concourse.bass_utils import run_bass_kernel_spmd
from concourse.masks import make_identity

P = 128
D = 1024
B = 4
T = 8192
VOCAB = 256
SENTINEL = 512.0  # out-of-range token -> one-hot col is all zero

W_SCAN = 128            # warmup scan steps
S_LOC = 1024 + W_SCAN   # scan steps computed per core (chunk 0 = warmup)
T_LOC = 4 * S_LOC       # raw steps per core (4608)
X_LOC = T_LOC + 8       # x slice incl conv halo (left 4, right 3, +1 pad)
N_TT = T_LOC // 512     # 9 T-tiles in stem
N_CH = S_LOC // 128     # 9 scan chunks
GROUPS = [(0, 3), (3, 3), (6, 3)]  # (first chunk, n chunks) per scan group
W_MAX = 384

f32 = mybir.dt.float32
bf16 = mybir.dt.bfloat16
AF = mybir.ActivationFunctionType
OP = mybir.AluOpType

# (conv_id, kernel_size, pad); tap offset = j - pad
CONVS = [(1, 0), (2, 1), (4, 2), (8, 4)]
TAPS = []  # (conv_id, j, off)
for ci, (K, pad) in enumerate(CONVS):
    for j in range(K):
        TAPS.append((ci, j, j - pad))
N_TAPS = len(TAPS)  # 15
TAPS_OF_CONV = [[kk for kk, (ci, _, _) in enumerate(TAPS) if ci == c] for c in range(4)]

_CACHE = {}


def _build():
    nc = bacc.Bacc()

    x_d = nc.declare_dram_parameter("x_loc", [X_LOC], bf16, isOutput=False)
    maskc_d = nc.declare_dram_parameter("mask_col", [P, 1], f32, isOutput=False)
    maskr_d = nc.declare_dram_parameter("mask_row", [W_MAX], bf16, isOutput=False)
    stem_d = nc.declare_dram_parameter("stem_w", [2, P, N_TAPS, 256], bf16, isOutput=False)
    convb_d = nc.declare_dram_parameter("convb", [P, 8], f32, isOutput=False)
    dw_d = nc.declare_dram_parameter("down_wt", [P, 4, 8, D], bf16, isOutput=False)
    downbr_d = nc.declare_dram_parameter("downb_v", [D], bf16, isOutput=False)
    bw_d = nc.declare_dram_parameter("b_wt", [P, 8, D], bf16, isOutput=False)
    bb2_d = nc.declare_dram_parameter("bb2", [P, 8], f32, isOutput=False)
    bb2r_d = nc.declare_dram_parameter("bb2_row", [D], bf16, isOutput=False)
    cw_d = nc.declare_dram_parameter("c_wt", [P, 8, D], bf16, isOutput=False)
    slnw_d = nc.declare_dram_parameter("slnw_v", [D], bf16, isOutput=False)
    ccb_d = nc.declare_dram_parameter("ccb_v", [D], bf16, isOutput=False)
    lruw_d = nc.declare_dram_parameter("lruw_v", [D], bf16, isOutput=False)
    lrub_d = nc.declare_dram_parameter("lrub_v", [D], bf16, isOutput=False)
    lam_d = nc.declare_dram_parameter("lam_ct", [P, 8], f32, isOutput=False)

    out_d = nc.declare_dram_parameter("out", [1024, D], bf16, isOutput=True)

    with tile.TileContext(nc) as tc:
        with tc.tile_pool(name="glob", bufs=1) as glob, \
             tc.tile_pool(name="pw", bufs=1) as pw, \
             tc.tile_pool(name="p12t", bufs=2) as p12t, \
             tc.tile_pool(name="p34t", bufs=2) as p34t, \
             tc.tile_pool(name="ps_stem", bufs=2, space="PSUM") as ps_stem, \
             tc.tile_pool(name="ps_down", bufs=2, space="PSUM") as ps_down, \
             tc.tile_pool(name="ps_tr", bufs=1, space="PSUM") as ps_tr, \
             tc.tile_pool(name="ps_bp", bufs=2, space="PSUM") as ps_bp, \
             tc.tile_pool(name="ps_cp", bufs=1, space="PSUM") as ps_cp:

            # ---------------- small global state -------------------------
            lam_sb = glob.tile([P, 8], f32, name="lam_sb")
            nc.sync.dma_start(lam_sb[:], lam_d[:])
            convb_sb = glob.tile([P, 8], f32, name="convb_sb")
            nc.sync.dma_start(convb_sb[:], convb_d[:])
            bb2_sb = glob.tile([P, 8], f32, name="bb2_sb")
            nc.sync.dma_start(bb2_sb[:], bb2_d[:])
            mask_col = glob.tile([P, 1], f32, name="mask_col")
            nc.sync.dma_start(mask_col[:], maskc_d[:])
            mask_row = glob.tile([1, W_MAX], bf16, name="mask_row")
            nc.sync.dma_start(mask_row[:], maskr_d[:][None, :])
            bb2_row = glob.tile([1, D], bf16, name="bb2_row")
            nc.sync.dma_start(bb2_row[:], bb2r_d[:][None, :])
            downb_rep = glob.tile([P, D], bf16, name="downb_rep")
            nc.sync.dma_start(downb_rep[:],
                              downbr_d[:][None, :].to_broadcast([P, D]))
            eps_sb = glob.tile([P, 1], f32, name="eps_sb")
            nc.vector.memset(eps_sb[:], 1e-5)
            ident = glob.tile([P, P], bf16, name="ident")
            make_identity(nc, ident)
            # ~4us of junk matmuls at kernel start: trips the PE HAM activity
            # window while the first weight DMAs land, so real matmuls start
            # at the full 2.4 GHz clock instead of the cold 1.2 GHz default.
            warm_ps = ps_cp.tile([P, 512], f32, name="psc", tag="psc")
            for wi in range(34):
                nc.tensor.matmul(warm_ps[:, :128], ident[:], ident[:],
                                 start=True, stop=True)
            io0 = glob.tile([P, 1], f32, name="io0")
            io1 = glob.tile([P, 1], f32, name="io1")
            nc.gpsimd.iota(io0[:], pattern=[[0, 1]], base=0, channel_multiplier=1,
                           allow_small_or_imprecise_dtypes=True)
            nc.gpsimd.iota(io1[:], pattern=[[0, 1]], base=128, channel_multiplier=1,
                           allow_small_or_imprecise_dtypes=True)

            # persistent activations
            z_bf = glob.tile([P, N_CH, D], bf16, name="z_bf")

            # big weights
            stem_sb0 = pw.tile([P, N_TAPS, 256], bf16, name="stem_sb0")
            stem_sb1 = pw.tile([P, N_TAPS, 256], bf16, name="stem_sb1")
            stem_sbs = (stem_sb0, stem_sb1)
            dw_sb = pw.tile([P, 4, 8, D], bf16, name="dw_sb")
            bw_sb = pw.tile([P, 8, D], bf16, name="bw_sb")
            cw_sb = pw.tile([P, 8, D], bf16, name="cw_sb")
            slnw_rep = pw.tile([P, D], bf16, name="slnw_rep")
            ccb_rep = pw.tile([P, D], bf16, name="ccb_rep")
            lruw_rep = pw.tile([P, D], bf16, name="lruw_rep")
            lrub_rep = pw.tile([P, D], bf16, name="lrub_rep")

            # first weight loads (stem tables + x windows before anything else)
            nc.sync.dma_start(stem_sb0[:], stem_d[0])

            x_reps = {}

            def issue_xrep(tt):
                x_rep = p12t.tile([P, 520], bf16, name="x_rep", bufs=2)
                nc.sync.dma_start(
                    x_rep[:],
                    x_d[tt * 512: tt * 512 + 520][None, :].to_broadcast([P, 520]))
                x_reps[tt] = x_rep

            issue_xrep(0)
            nc.sync.dma_start(stem_sb1[:], stem_d[1])
            issue_xrep(1)

            hm_ts = {}

            def stem(tt):
                x_rep = x_reps.pop(tt)
                oh = p12t.tile([P, 2, 520], bf16, name="oh", bufs=2)
                nc.vector.tensor_scalar(out=oh[:, 0, :], in0=x_rep[:],
                                        scalar1=io0[:], scalar2=None,
                                        op0=OP.is_equal)
                nc.vector.tensor_scalar(out=oh[:, 1, :], in0=x_rep[:],
                                        scalar1=io1[:], scalar2=None,
                                        op0=OP.is_equal)
                hm_t = p12t.tile([P, 8, 512], bf16, name="hm_t", bufs=2)
                hm_ts[tt] = hm_t
                for cc in range(8):
                    ci, half = cc // 2, cc % 2
                    taps = TAPS_OF_CONV[ci]
                    ps = ps_stem.tile([P, 512], f32, name="pss", tag="pss")
                    n_mm = len(taps) * 2
                    i = 0
                    for vc in range(2):
                        for kk in taps:
                            off = TAPS[kk][2]
                            nc.tensor.matmul(
                                ps[:],
                                stem_sbs[vc][:, kk, half * 128:(half + 1) * 128],
                                oh[:, vc, 4 + off: 4 + off + 512],
                                start=(i == 0), stop=(i == n_mm - 1))
                            i += 1
                    nc.scalar.activation(hm_t[:, cc, :], ps[:], AF.Gelu,
                                         bias=convb_sb[:, cc:cc + 1])

            hd_ts = {}

            def down(c):
                hm_sb = hm_ts.pop(c)
                hd_t = p34t.tile([P, D], bf16, name="hd_t", tag="hd", bufs=2)
                hd_ts[c] = hd_t
                for eh in range(2):
                    ps = ps_down.tile([P, 512], f32, name="psd", tag="psd")
                    i = 0
                    for dc in range(8):
                        for j in range(4):
                            nc.tensor.matmul(
                                ps[:],
                                hm_sb[:, dc, j:512:4],
                                dw_sb[:, j, dc, eh * 512:(eh + 1) * 512],
                                start=(i == 0), stop=(i == 31))
                            i += 1
                    # psum -> bf16 hd with the per-e down bias, on DVE
                    nc.vector.tensor_tensor(
                        out=hd_t[:, eh * 512:(eh + 1) * 512], in0=ps[:],
                        in1=downb_rep[:, eh * 512:(eh + 1) * 512], op=OP.add)

            def lnt(c):
                """LN stats + z + transpose for chunk c."""
                g, lc = next((gi, c - g0) for gi, (g0, gn) in enumerate(GROUPS)
                             if g0 <= c < g0 + gn)
                hd_t = hd_ts.pop(c)
                stats = p34t.tile([P, 2, 6], f32, name="stats", bufs=2)
                hd_g = hd_t[:].rearrange("p (g f) -> p g f", g=2)
                nc.vector.bn_stats(out=stats[:, 0, :], in_=hd_g[:, 0, :])
                nc.vector.bn_stats(out=stats[:, 1, :], in_=hd_g[:, 1, :])
                mv = p34t.tile([P, 2], f32, name="mv", bufs=2)
                nc.vector.bn_aggr(out=mv[:], in_=stats[:])
                rstd = p34t.tile([P, 1], f32, name="rstd", bufs=2)
                nc.scalar.activation(rstd[:], mv[:, 1:2], AF.Sqrt, bias=eps_sb[:])
                nc.vector.reciprocal(rstd[:], rstd[:])
                if c == 0:
                    # zero warmup z on h=0 cores (mask) by zeroing rstd
                    nc.vector.tensor_tensor(out=rstd[:], in0=rstd[:],
                                            in1=mask_col[:], op=OP.mult)
                nc.vector.tensor_scalar(out=z_bf[:, c, :], in0=hd_t[:],
                                        scalar1=mv[:, 0:1], scalar2=rstd[:],
                                        op0=OP.subtract, op1=OP.mult)
                pst = ps_tr.tile([P, 8, P], bf16, name="pst", tag="pst")
                for ec in range(8):
                    nc.tensor.transpose(
                        pst[:, ec, :], z_bf[:, c, ec * 128:(ec + 1) * 128],
                        ident[:])
                hsT_g = hsT_tiles[g]
                nc.scalar.copy(hsT_g[:, :, lc * 128:(lc + 1) * 128], pst[:])

            hsT_tiles = {}
            h_tiles = {}

            def open_group(g):
                hsT_tiles[g] = p34t.tile([P, 8, W_MAX], bf16, name="hsT",
                                         tag="hsT", bufs=2)

            def bproj_scan(g):
                g0, gn = GROUPS[g]
                W = gn * 128
                hsT_g = hsT_tiles[g]
                h_g = p34t.tile([P, 8, W_MAX], bf16, name="h_g", tag="h_g",
                                bufs=2)
                h_tiles[g] = h_g
                for dc in range(8):
                    psb = ps_bp.tile([P, W_MAX], f32, name="psb", tag="psb")
                    if g == 0:
                        # masked per-channel bias via 1-row matmul
                        nc.tensor.matmul(psb[:, :W],
                                         bb2_row[:, dc * 128:(dc + 1) * 128],
                                         mask_row[:, :W],
                                         start=True, stop=False)
                    for ec in range(8):
                        nc.tensor.matmul(
                            psb[:, :W],
                            bw_sb[:, ec, dc * 128:(dc + 1) * 128],
                            hsT_g[:, ec, :W],
                            start=(g != 0 and ec == 0), stop=(ec == 7))
                    vals = p34t.tile([P, W_MAX], bf16, name="vals", bufs=1)
                    if g == 0:
                        # bias already in psum (masked); plain copy
                        nc.vector.tensor_scalar(out=vals[:, :W],
                                                in0=psb[:, :W], scalar1=0.0,
                                                scalar2=None, op0=OP.add)
                    else:
                        nc.vector.tensor_scalar(out=vals[:, :W],
                                                in0=psb[:, :W],
                                                scalar1=bb2_sb[:, dc:dc + 1],
                                                scalar2=None, op0=OP.add)
                    init = (0.0 if g == 0
                            else h_tiles[g - 1][:, dc,
                                                GROUPS[g - 1][1] * 128 - 1:
                                                GROUPS[g - 1][1] * 128])
                    nc.vector.tensor_tensor_scan(
                        out=h_g[:, dc, :W],
                        data0=lam_sb[:, dc:dc + 1].to_broadcast([P, W]),
                        data1=vals[:, :W],
                        initial=init, op0=OP.mult, op1=OP.add)

            def p4(c):
                """c-proj + residual + final LN -> out rows (c-1)*128.."""
                g, lc = next((gi, c - g0) for gi, (g0, gn) in enumerate(GROUPS)
                             if g0 <= c < g0 + gn)
                h_g = h_tiles[g]
                res_b = p34t.tile([P, D], bf16, name="res_b", bufs=2)
                nc.gpsimd.tensor_tensor(out=res_b[:], in0=z_bf[:, c, :],
                                        in1=slnw_rep[:], op=OP.mult)
                nc.gpsimd.tensor_tensor(out=res_b[:], in0=res_b[:],
                                        in1=ccb_rep[:], op=OP.add)
                res_f = p34t.tile([P, D], bf16, name="res_f", bufs=2)
                for eh in range(2):
                    psc = ps_cp.tile([P, 512], f32, name="psc", tag="psc")
                    for dc in range(8):
                        nc.tensor.matmul(
                            psc[:],
                            h_g[:, dc, lc * 128:(lc + 1) * 128],
                            cw_sb[:, dc, eh * 512:(eh + 1) * 512],
                            start=(dc == 0), stop=(dc == 7))
                    nc.vector.tensor_tensor(
                        out=res_f[:, eh * 512:(eh + 1) * 512], in0=psc[:],
                        in1=res_b[:, eh * 512:(eh + 1) * 512], op=OP.add)
                stats2 = p34t.tile([P, 2, 6], f32, name="stats2", bufs=2)
                res_g = res_f[:].rearrange("p (g f) -> p g f", g=2)
                nc.vector.bn_stats(out=stats2[:, 0, :], in_=res_g[:, 0, :])
                nc.vector.bn_stats(out=stats2[:, 1, :], in_=res_g[:, 1, :])
                mv2 = p34t.tile([P, 2], f32, name="mv2", bufs=2)
                nc.vector.bn_aggr(out=mv2[:], in_=stats2[:])
                rstd2 = p34t.tile([P, 1], f32, name="rstd2", bufs=2)
                nc.scalar.activation(rstd2[:], mv2[:, 1:2], AF.Sqrt,
                                     bias=eps_sb[:])
                nc.vector.reciprocal(rstd2[:], rstd2[:])
                nc.vector.tensor_scalar(out=res_f[:], in0=res_f[:],
                                        scalar1=mv2[:, 0:1], scalar2=rstd2[:],
                                        op0=OP.subtract, op1=OP.mult)
                nc.vector.tensor_tensor(out=res_f[:], in0=res_f[:],
                                        in1=lruw_rep[:], op=OP.mult)
                nc.vector.tensor_tensor(out=res_f[:], in0=res_f[:],
                                        in1=lrub_rep[:], op=OP.add)
                nc.sync.dma_start(out_d[(c - 1) * 128: c * 128, :], res_f[:])

            # ---------------- software-pipelined emission ----------------
            open_group(0)
            stem(0)
            issue_xrep(2)
            nc.sync.dma_start(dw_sb[:, 0, :, :], dw_d[:, 0, :, :])
            nc.sync.dma_start(dw_sb[:, 1, :, :], dw_d[:, 1, :, :])
            stem(1)
            issue_xrep(3)
            nc.sync.dma_start(dw_sb[:, 2, :, :], dw_d[:, 2, :, :])
            nc.sync.dma_start(dw_sb[:, 3, :, :], dw_d[:, 3, :, :])
            stem(2)
            issue_xrep(4)
            down(0)
            nc.sync.dma_start(bw_sb[:], bw_d[:])
            down(1)
            lnt(0)
            stem(3)
            issue_xrep(5)
            down(2)
            lnt(1)
            nc.sync.dma_start(slnw_rep[:], slnw_d[:][None, :].to_broadcast([P, D]))
            nc.sync.dma_start(ccb_rep[:], ccb_d[:][None, :].to_broadcast([P, D]))
            stem(4)
            issue_xrep(6)
            down(3)
            lnt(2)
            bproj_scan(0)
            open_group(1)
            nc.sync.dma_start(cw_sb[:], cw_d[:])
            nc.sync.dma_start(lruw_rep[:], lruw_d[:][None, :].to_broadcast([P, D]))
            nc.sync.dma_start(lrub_rep[:], lrub_d[:][None, :].to_broadcast([P, D]))
            stem(5)
            issue_xrep(7)
            down(4)
            lnt(3)
            p4(1)
            stem(6)
            issue_xrep(8)
            down(5)
            lnt(4)
            p4(2)
            stem(7)
            down(6)
            lnt(5)
            bproj_scan(1)
            open_group(2)
            stem(8)
            down(7)
            lnt(6)
            lnt(7)
            p4(3)
            p4(4)
            down(8)
            lnt(8)
            p4(5)
            bproj_scan(2)
            p4(6)
            p4(7)
            p4(8)

    nc.finalize()
    return nc


def _prep_host(inputs):
    import ml_dtypes
    f = np.float32
    bf = ml_dtypes.bfloat16
    embed = np.asarray(inputs["embed"], f)
    conv_ws = [np.asarray(inputs[k], f) for k in
               ("conv1_w", "conv2_w", "conv4_w", "conv8_w")]
    conv_bs = [np.asarray(inputs[k], f) for k in
               ("conv1_b", "conv2_b", "conv4_b", "conv8_b")]
    down_w = np.asarray(inputs["down_w"], f)
    log_lam = np.asarray(inputs["log_lambda_raw"], f)
    lam = (1.0 / (1.0 + np.exp(-log_lam.astype(np.float64)))).astype(f)
    b_w = np.asarray(inputs["b_w"], f)
    c_w = np.asarray(inputs["c_w"], f)

    stem_w = np.empty((2, P, N_TAPS, 256), f)
    for kk, (ci, j, _off) in enumerate(TAPS):
        fused = embed @ conv_ws[ci][:, :, j].T        # [256v, 256c]
        stem_w[:, :, kk, :] = fused.reshape(2, P, 256)
    convb = np.concatenate(conv_bs).reshape(8, P).T.copy()      # [p, cc]

    down_wt = (down_w.transpose(1, 2, 0)                        # [d, j, e]
               .reshape(8, P, 4, D).transpose(1, 2, 0, 3).copy())  # [p, j, dc, e]
    one_m = (1.0 - lam)
    slnw = np.asarray(inputs["stem_ln_w"], f)
    slnb = np.asarray(inputs["stem_ln_b"], f)
    # values[d,t] = sum_e [(1-lam_d) b_w[d,e] slnw[e]] z^T[e,t]
    #              + (1-lam_d)(b_w[d,:] @ slnb + b_b[d])
    b_wt = ((b_w.T * one_m[None, :] * slnw[:, None])            # [e, d]
            .reshape(8, P, D).transpose(1, 0, 2).copy())        # [p, ec, d]
    bb2 = (one_m * (b_w @ slnb + np.asarray(inputs["b_b"], f))
           ).reshape(8, P).T.copy()                             # [p, dc]
    bb2_row = (one_m * (b_w @ slnb + np.asarray(inputs["b_b"], f)))  # [d]
    c_wt = c_w.T.reshape(8, P, D).transpose(1, 0, 2).copy()     # [p, dc, e]
    lam_ct = lam.reshape(8, P).T.copy()
    ccb = slnb + np.asarray(inputs["c_b"], f)

    shared = dict(
        stem_w=stem_w.astype(bf), convb=convb,
        down_wt=down_wt.astype(bf),
        downb_v=np.asarray(inputs["down_b"], f).astype(bf),
        b_wt=b_wt.astype(bf), bb2=bb2, bb2_row=bb2_row.astype(bf),
        c_wt=c_wt.astype(bf),
        slnw_v=slnw.astype(bf), ccb_v=ccb.astype(bf),
        lruw_v=np.asarray(inputs["lru_ln_w"], f).astype(bf),
        lrub_v=np.asarray(inputs["lru_ln_b"], f).astype(bf),
        lam_ct=lam_ct,
    )

    x = np.asarray(inputs["x"]).astype(np.int64)
    in_maps = []
    for core in range(8):
        b, h = core // 2, core % 2
        t0 = h * 4096
        idx = t0 - 516 + np.arange(X_LOC)
        valid = (idx >= 0) & (idx < T)
        x_loc = np.full((X_LOC,), SENTINEL, bf)
        x_loc[valid] = x[b, idx[valid]].astype(bf)
        mask = np.ones((S_LOC,), f)
        if h == 0:
            mask[:W_SCAN] = 0.0
        m = dict(shared)
        m["x_loc"] = x_loc
        m["mask_col"] = mask[:P].reshape(P, 1).copy()
        m["mask_row"] = mask[:W_MAX].astype(bf)
        in_maps.append(m)
    return in_maps


def kernel(**inputs) -> np.ndarray:
    if "nc" not in _CACHE:
        _CACHE["nc"] = _build()
    nc = _CACHE["nc"]
    in_maps = _prep_host(inputs)
    res = run_bass_kernel_spmd(nc, in_maps, list(range(8)))
    out = np.empty((B, 2048, D), np.float32)
    for core in range(8):
        b, h = core // 2, core % 2
        out[b, h * 1024:(h + 1) * 1024, :] = np.asarray(
            res.results[core]["out"], np.float32)
    return out


# revision 30
# speedup vs baseline: 1.6395x; 1.0629x over previous
"""Trainium2 Bass kernel for nn_ByteEncoder (multi-scale conv stem + per-channel LRU).

Sharding: 8 cores = (batch b in 0..3) x (time-half h in 0..1). Each core runs an
identical SPMD program over raw steps [t0-128, t0+4096) (t0 = h*4096), i.e. a
32-scan-step warmup plus its 1024 output scan steps. The warmup region is
masked to zero for h=0 cores (reference scan starts at state 0) and uses real
left-context for h=1 cores (per-channel decay lambda^32 < 1.5e-6, far below the
2e-2 tolerance).

The embedding lookup is algebraically fused into the conv stem: for one-hot
inputs, conv_k(embed[x]) == sum_taps (embed @ conv_w[:,:,j])[x[t+off]], so the
stem becomes matmuls of precontracted [256-vocab x 256-ch] tables against
one-hot columns built on-chip (iota + is_equal).

All matmuls run in bf16 (full PE rate, fast-weight-load path, half the SBUF and
HBM traffic of fp32). Everything stays in SBUF: stem and the strided down-conv
are fused per 512-step tile, and phase 3/4 (LN -> b-proj -> scan -> c-proj ->
LN) interleaves with later stem tiles so the tensor engine never idles. The
short warmup tile computes its down-conv transposed (cheap N=32 matmuls) and
normalizes across partitions via ones-matmul stats + partition_broadcast.
"""
import numpy as np

import concourse.bass as bass
import concourse.tile as tile
from concourse import mybir, bacc
from concourse.bass_utils import run_bass_kernel_spmd
from concourse.masks import make_identity

P = 128
D = 1024
B = 4
T = 8192
VOCAB = 256
SENTINEL = 512.0  # out-of-range token -> one-hot col is all zero

W_SCAN = 32             # warmup scan steps (lam^32 < 1.5e-6)
S_LOC = 1024 + W_SCAN   # scan steps computed per core
T_LOC = 4 * S_LOC       # raw steps per core (4224)
X_LOC = T_LOC + 8       # x slice incl conv halo (left 4, right 3, +1 pad)
N_CH = 8                # output chunks c1..c8 (128 scan steps each)
W_MAX = 384

f32 = mybir.dt.float32
bf16 = mybir.dt.bfloat16
AF = mybir.ActivationFunctionType
OP = mybir.AluOpType

# scan groups, in scan columns: g0 = warmup(32) + chunks 1-2, then 3/2/1 chunks.
# Last group is a single chunk so the serial tail after the last stem tile is
# as short as possible.
GROUPS = [(32 + 256), 384, 256, 128]
# chunk k (1..8) -> (group, col offset inside group)
CHUNK_POS = {1: (0, 32), 2: (0, 160), 3: (1, 0), 4: (1, 128), 5: (1, 256),
             6: (2, 0), 7: (2, 128), 8: (3, 0)}

# (conv_id, kernel_size, pad); tap offset = j - pad
CONVS = [(1, 0), (2, 1), (4, 2), (8, 4)]
TAPS = []  # (conv_id, j, off)
for ci, (K, pad) in enumerate(CONVS):
    for j in range(K):
        TAPS.append((ci, j, j - pad))
N_TAPS = len(TAPS)  # 15
TAPS_OF_CONV = [[kk for kk, (ci, _, _) in enumerate(TAPS) if ci == c] for c in range(4)]

_CACHE = {}


def _build():
    nc = bacc.Bacc()

    x_d = nc.declare_dram_parameter("x_loc", [X_LOC], bf16, isOutput=False)
    maskr_d = nc.declare_dram_parameter("mask_row", [W_MAX], bf16, isOutput=False)
    stem_d = nc.declare_dram_parameter("stem_w", [2, P, N_TAPS, 256], bf16, isOutput=False)
    convb_d = nc.declare_dram_parameter("convb", [P, 8], f32, isOutput=False)
    dw_d = nc.declare_dram_parameter("down_wt", [P, 4, 8, D], bf16, isOutput=False)
    downbr_d = nc.declare_dram_parameter("downb_v", [D], bf16, isOutput=False)
    downbT_d = nc.declare_dram_parameter("downbT", [P, 8], f32, isOutput=False)
    bw_d = nc.declare_dram_parameter("b_wt", [P, 8, D], bf16, isOutput=False)
    bb2_d = nc.declare_dram_parameter("bb2", [P, 8], f32, isOutput=False)
    bb2r_d = nc.declare_dram_parameter("bb2_row", [D], bf16, isOutput=False)
    cw_d = nc.declare_dram_parameter("c_wt", [P, 8, D], bf16, isOutput=False)
    slnw_d = nc.declare_dram_parameter("slnw_v", [D], bf16, isOutput=False)
    ccb_d = nc.declare_dram_parameter("ccb_v", [D], bf16, isOutput=False)
    lruw_d = nc.declare_dram_parameter("lruw_v", [D], bf16, isOutput=False)
    lrub_d = nc.declare_dram_parameter("lrub_v", [D], bf16, isOutput=False)
    lam_d = nc.declare_dram_parameter("lam_ct", [P, 8], f32, isOutput=False)

    out_d = nc.declare_dram_parameter("out", [1024, D], bf16, isOutput=True)

    with tile.TileContext(nc) as tc:
        with tc.tile_pool(name="glob", bufs=1) as glob, \
             tc.tile_pool(name="pw", bufs=1) as pw, \
             tc.tile_pool(name="p12t", bufs=2) as p12t, \
             tc.tile_pool(name="p34t", bufs=2) as p34t, \
             tc.tile_pool(name="ps_stem", bufs=2, space="PSUM") as ps_stem, \
             tc.tile_pool(name="ps_down", bufs=2, space="PSUM") as ps_down, \
             tc.tile_pool(name="ps_tr", bufs=1, space="PSUM") as ps_tr, \
             tc.tile_pool(name="ps_bp", bufs=2, space="PSUM") as ps_bp, \
             tc.tile_pool(name="ps_cp", bufs=1, space="PSUM") as ps_cp:

            # ---------------- on-chip constants (no DMA) -----------------
            eps_sb = glob.tile([P, 1], f32, name="eps_sb")
            nc.vector.memset(eps_sb[:], 1e-5)
            ones128 = glob.tile([P, 1], bf16, name="ones128")
            nc.vector.memset(ones128[:], 1.0)
            ident = glob.tile([P, P], bf16, name="ident")
            make_identity(nc, ident)
            io0 = glob.tile([P, 1], f32, name="io0")
            io1 = glob.tile([P, 1], f32, name="io1")
            nc.gpsimd.iota(io0[:], pattern=[[0, 1]], base=0, channel_multiplier=1,
                           allow_small_or_imprecise_dtypes=True)
            nc.gpsimd.iota(io1[:], pattern=[[0, 1]], base=128, channel_multiplier=1,
                           allow_small_or_imprecise_dtypes=True)
            # ~4.5us of junk matmuls at kernel start: trips the PE HAM
            # activity window while the first weight DMAs land, so real
            # matmuls start at the full 2.4 GHz clock.
            warm_ps = ps_cp.tile([P, 512], f32, name="psc", tag="psc")
            for _ in range(34):
                nc.tensor.matmul(warm_ps[:, :128], ident[:], ident[:],
                                 start=True, stop=True)

            # ---------------- critical-path DMAs first -------------------
            stem_sb0 = pw.tile([P, N_TAPS, 256], bf16, name="stem_sb0")
            stem_sb1 = pw.tile([P, N_TAPS, 256], bf16, name="stem_sb1")
            stem_sbs = (stem_sb0, stem_sb1)
            nc.sync.dma_start(stem_sb0[:], stem_d[0])
            convb_sb = glob.tile([P, 8], f32, name="convb_sb")
            nc.sync.dma_start(convb_sb[:], convb_d[:])

            x_reps = {}

            def issue_xrep(tt):
                x_rep = p12t.tile([P, 520], bf16, name="x_rep", bufs=2)
                if tt == 0:
                    nc.sync.dma_start(
                        x_rep[:, :136],
                        x_d[0:136][None, :].to_broadcast([P, 136]))
                else:
                    lo = 512 * tt - 384
                    nc.sync.dma_start(
                        x_rep[:],
                        x_d[lo: lo + 520][None, :].to_broadcast([P, 520]))
                x_reps[tt] = x_rep

            issue_xrep(0)
            nc.sync.dma_start(stem_sb1[:], stem_d[1])
            issue_xrep(1)

            # ---------------- remaining SBUF state -----------------------
            z_bf = glob.tile([P, N_CH, D], bf16, name="z_bf")
            lam_sb = glob.tile([P, 8], f32, name="lam_sb")
            bb2_sb = glob.tile([P, 8], f32, name="bb2_sb")
            mask_row = glob.tile([1, W_MAX], bf16, name="mask_row")
            bb2_row = glob.tile([1, D], bf16, name="bb2_row")
            downb_rep = glob.tile([P, D], bf16, name="downb_rep")
            dw_sb = pw.tile([P, 4, 8, D], bf16, name="dw_sb")
            bw_sb = pw.tile([P, 8, D], bf16, name="bw_sb")
            cw_sb = pw.tile([P, 8, D], bf16, name="cw_sb")
            slnw_rep = pw.tile([P, D], bf16, name="slnw_rep")
            ccb_rep = pw.tile([P, D], bf16, name="ccb_rep")
            lruw_rep = pw.tile([P, D], bf16, name="lruw_rep")
            lrub_rep = pw.tile([P, D], bf16, name="lrub_rep")

            hm_ts = {}
            hm0 = p12t.tile([P, 8, 128], bf16, name="hm0", bufs=1)

            def stem(tt):
                wid = 128 if tt == 0 else 512
                x_rep = x_reps.pop(tt)
                oh = p12t.tile([P, 2, 520], bf16, name="oh", bufs=2)
                nc.vector.tensor_scalar(out=oh[:, 0, :wid + 8],
                                        in0=x_rep[:, :wid + 8],
                                        scalar1=io0[:], scalar2=None,
                                        op0=OP.is_equal)
                nc.vector.tensor_scalar(out=oh[:, 1, :wid + 8],
                                        in0=x_rep[:, :wid + 8],
                                        scalar1=io1[:], scalar2=None,
                                        op0=OP.is_equal)
                if tt == 0:
                    hm_t = hm0
                else:
                    hm_t = p12t.tile([P, 8, 512], bf16, name="hm_t", bufs=2)
                    hm_ts[tt] = hm_t
                for cc in range(8):
                    ci, half = cc // 2, cc % 2
                    taps = TAPS_OF_CONV[ci]
                    ps = ps_stem.tile([P, 512], f32, name="pss", tag="pss")
                    n_mm = len(taps) * 2
                    i = 0
                    for vc in range(2):
                        for kk in taps:
                            off = TAPS[kk][2]
                            nc.tensor.matmul(
                                ps[:, :wid],
                                stem_sbs[vc][:, kk, half * 128:(half + 1) * 128],
                                oh[:, vc, 4 + off: 4 + off + wid],
                                start=(i == 0), stop=(i == n_mm - 1))
                            i += 1
                    nc.scalar.activation(hm_t[:, cc, :wid], ps[:, :wid],
                                         AF.Gelu, bias=convb_sb[:, cc:cc + 1])

            hd_ts = {}

            def down(c):
                """Standard down-conv for chunk c (tile c), c = 1..8."""
                hm_sb = hm_ts.pop(c)
                hd_t = p34t.tile([P, D], bf16, name="hd_t", tag="hd", bufs=2)
                hd_ts[c] = hd_t
                for eh in range(2):
                    ps = ps_down.tile([P, 512], f32, name="psd", tag="psd")
                    i = 0
                    for dc in range(8):
                        for j in range(4):
                            nc.tensor.matmul(
                                ps[:],
                                hm_sb[:, dc, j:512:4],
                                dw_sb[:, j, dc, eh * 512:(eh + 1) * 512],
                                start=(i == 0), stop=(i == 31))
                            i += 1
                    nc.vector.tensor_tensor(
                        out=hd_t[:, eh * 512:(eh + 1) * 512], in0=ps[:],
                        in1=downb_rep[:, eh * 512:(eh + 1) * 512], op=OP.add)

            def down0_ln0():
                """Warmup (32 scan steps): transposed down-conv (out [e, s],
                cheap N=32 matmuls), partition-axis LN via ones-matmul stats,
                result written straight into hsT group 0 cols 0:32."""
                hdT0 = p34t.tile([P, 8, 32], bf16, name="hdT0", bufs=1)
                for eb in range(8):
                    ps = ps_down.tile([P, 512], f32, name="psd", tag="psd")
                    i = 0
                    for dc in range(8):
                        for j in range(4):
                            nc.tensor.matmul(
                                ps[:128, :32],
                                dw_sb[:, j, dc, eb * 128:(eb + 1) * 128],
                                hm0[:, dc, j:128:4],
                                start=(i == 0), stop=(i == 31))
                            i += 1
                    # add down bias for these 128 e-channels, transposed:
                    # downb slice as per-partition scalar
                    nc.vector.tensor_scalar(
                        out=hdT0[:, eb, :], in0=ps[:128, :32],
                        scalar1=downbT[:, eb:eb + 1], scalar2=None, op0=OP.add)
                sq0 = p34t.tile([P, 8, 32], bf16, name="sq0", bufs=1)
                nc.vector.tensor_tensor(out=sq0[:], in0=hdT0[:], in1=hdT0[:],
                                        op=OP.mult)
                ps = ps_bp.tile([P, W_MAX], f32, name="psb", tag="psb")
                for eb in range(8):
                    nc.tensor.matmul(ps[0:1, 0:32], ones128[:], hdT0[:, eb, :],
                                     start=(eb == 0), stop=(eb == 7))
                for eb in range(8):
                    nc.tensor.matmul(ps[0:1, 64:96], ones128[:], sq0[:, eb, :],
                                     start=(eb == 0), stop=(eb == 7))
                m_row = p34t.tile([1, 32], f32, name="m_row", bufs=1)
                nc.vector.tensor_scalar(out=m_row[:], in0=ps[0:1, 0:32],
                                        scalar1=1.0 / D, scalar2=None,
                                        op0=OP.mult)
                v_row = p34t.tile([1, 32], f32, name="v_row", bufs=1)
                nc.vector.tensor_scalar(out=v_row[:], in0=ps[0:1, 64:96],
                                        scalar1=1.0 / D, scalar2=None,
                                        op0=OP.mult)
                msq = p34t.tile([1, 32], f32, name="msq", bufs=1)
                nc.vector.tensor_tensor(out=msq[:], in0=m_row[:], in1=m_row[:],
                                        op=OP.mult)
                nc.vector.tensor_tensor(out=v_row[:], in0=v_row[:], in1=msq[:],
                                        op=OP.subtract)
                nc.scalar.activation(v_row[:], v_row[:], AF.Sqrt,
                                     bias=eps_sb[0:1, :])
                nc.vector.reciprocal(v_row[:], v_row[:])
                # zero the warmup on h=0 cores
                nc.vector.tensor_tensor(out=v_row[:], in0=v_row[:],
                                        in1=mask_row[:, 0:32], op=OP.mult)
                m_rep = p34t.tile([P, 32], f32, name="m_rep", bufs=1)
                r_rep = p34t.tile([P, 32], f32, name="r_rep", bufs=1)
                nc.gpsimd.partition_broadcast(m_rep[:], m_row[:])
                nc.gpsimd.partition_broadcast(r_rep[:], v_row[:])
                hsT_g = hsT_tiles[0]
                for eb in range(8):
                    zt = p34t.tile([P, 32], f32, name="zt", bufs=2)
                    nc.vector.tensor_tensor(out=zt[:], in0=hdT0[:, eb, :],
                                            in1=m_rep[:], op=OP.subtract)
                    nc.vector.tensor_tensor(out=hsT_g[:, eb, 0:32], in0=zt[:],
                                            in1=r_rep[:], op=OP.mult)

            def lnt(k):
                """LN stats + z + transpose for chunk k (k = 1..8)."""
                g, off = CHUNK_POS[k]
                hd_t = hd_ts.pop(k)
                stats = p34t.tile([P, 2, 6], f32, name="stats", bufs=2)
                hd_g = hd_t[:].rearrange("p (g f) -> p g f", g=2)
                nc.vector.bn_stats(out=stats[:, 0, :], in_=hd_g[:, 0, :])
                nc.vector.bn_stats(out=stats[:, 1, :], in_=hd_g[:, 1, :])
                mv = p34t.tile([P, 2], f32, name="mv", bufs=2)
                nc.vector.bn_aggr(out=mv[:], in_=stats[:])
                rstd = p34t.tile([P, 1], f32, name="rstd", bufs=2)
                nc.scalar.activation(rstd[:], mv[:, 1:2], AF.Sqrt, bias=eps_sb[:])
                nc.vector.reciprocal(rstd[:], rstd[:])
                nc.vector.tensor_scalar(out=z_bf[:, k - 1, :], in0=hd_t[:],
                                        scalar1=mv[:, 0:1], scalar2=rstd[:],
                                        op0=OP.subtract, op1=OP.mult)
                pst = ps_tr.tile([P, 8, P], bf16, name="pst", tag="pst")
                for ec in range(8):
                    nc.tensor.transpose(
                        pst[:, ec, :], z_bf[:, k - 1, ec * 128:(ec + 1) * 128],
                        ident[:])
                hsT_g = hsT_tiles[g]
                nc.scalar.copy(hsT_g[:, :, off:off + 128], pst[:])

            hsT_tiles = {}
            h_tiles = {}

            def open_group(g):
                hsT_tiles[g] = p34t.tile([P, 8, W_MAX], bf16, name="hsT",
                                         tag="hsT", bufs=2)

            def bproj_scan(g):
                W = GROUPS[g]
                hsT_g = hsT_tiles[g]
                h_g = p34t.tile([P, 8, W_MAX], bf16, name="h_g", tag="h_g",
                                bufs=2)
                h_tiles[g] = h_g
                for dc in range(8):
                    psb = ps_bp.tile([P, W_MAX], f32, name="psb", tag="psb")
                    if g == 0:
                        # masked per-channel bias via 1-row matmul
                        nc.tensor.matmul(psb[:, :W],
                                         bb2_row[:, dc * 128:(dc + 1) * 128],
                                         mask_row[:, :W],
                                         start=True, stop=False)
                    for ec in range(8):
                        nc.tensor.matmul(
                            psb[:, :W],
                            bw_sb[:, ec, dc * 128:(dc + 1) * 128],
                            hsT_g[:, ec, :W],
                            start=(g != 0 and ec == 0), stop=(ec == 7))
                    vals = p34t.tile([P, W_MAX], bf16, name="vals", bufs=1)
                    if g == 0:
                        nc.vector.tensor_scalar(out=vals[:, :W],
                                                in0=psb[:, :W], scalar1=0.0,
                                                scalar2=None, op0=OP.add)
                    else:
                        nc.vector.tensor_scalar(out=vals[:, :W],
                                                in0=psb[:, :W],
                                                scalar1=bb2_sb[:, dc:dc + 1],
                                                scalar2=None, op0=OP.add)
                    init = (0.0 if g == 0
                            else h_tiles[g - 1][:, dc,
                                                GROUPS[g - 1] - 1: GROUPS[g - 1]])
                    nc.vector.tensor_tensor_scan(
                        out=h_g[:, dc, :W],
                        data0=lam_sb[:, dc:dc + 1].to_broadcast([P, W]),
                        data1=vals[:, :W],
                        initial=init, op0=OP.mult, op1=OP.add)

            def p4(k):
                """c-proj + residual + final LN -> out rows (k-1)*128.."""
                g, off = CHUNK_POS[k]
                h_g = h_tiles[g]
                res_b = p34t.tile([P, D], bf16, name="res_b", bufs=2)
                nc.gpsimd.tensor_tensor(out=res_b[:], in0=z_bf[:, k - 1, :],
                                        in1=slnw_rep[:], op=OP.mult)
                nc.gpsimd.tensor_tensor(out=res_b[:], in0=res_b[:],
                                        in1=ccb_rep[:], op=OP.add)
                res_f = p34t.tile([P, D], bf16, name="res_f", bufs=2)
                for eh in range(2):
                    psc = ps_cp.tile([P, 512], f32, name="psc", tag="psc")
                    for dc in range(8):
                        nc.tensor.matmul(
                            psc[:],
                            h_g[:, dc, off:off + 128],
                            cw_sb[:, dc, eh * 512:(eh + 1) * 512],
                            start=(dc == 0), stop=(dc == 7))
                    nc.vector.tensor_tensor(
                        out=res_f[:, eh * 512:(eh + 1) * 512], in0=psc[:],
                        in1=res_b[:, eh * 512:(eh + 1) * 512], op=OP.add)
                stats2 = p34t.tile([P, 2, 6], f32, name="stats2", bufs=2)
                res_g = res_f[:].rearrange("p (g f) -> p g f", g=2)
                nc.vector.bn_stats(out=stats2[:, 0, :], in_=res_g[:, 0, :])
                nc.vector.bn_stats(out=stats2[:, 1, :], in_=res_g[:, 1, :])
                mv2 = p34t.tile([P, 2], f32, name="mv2", bufs=2)
                nc.vector.bn_aggr(out=mv2[:], in_=stats2[:])
                rstd2 = p34t.tile([P, 1], f32, name="rstd2", bufs=2)
                nc.scalar.activation(rstd2[:], mv2[:, 1:2], AF.Sqrt,
                                     bias=eps_sb[:])
                nc.vector.reciprocal(rstd2[:], rstd2[:])
                nc.vector.tensor_scalar(out=res_f[:], in0=res_f[:],
                                        scalar1=mv2[:, 0:1], scalar2=rstd2[:],
                                        op0=OP.subtract, op1=OP.mult)
                nc.vector.tensor_tensor(out=res_f[:], in0=res_f[:],
                                        in1=lruw_rep[:], op=OP.mult)
                nc.vector.tensor_tensor(out=res_f[:], in0=res_f[:],
                                        in1=lrub_rep[:], op=OP.add)
                nc.sync.dma_start(out_d[(k - 1) * 128: k * 128, :], res_f[:])

            # ---------------- software-pipelined emission ----------------
            open_group(0)
            stem(0)
            issue_xrep(2)
            nc.sync.dma_start(dw_sb[:, 0, :, :], dw_d[:, 0, :, :])
            nc.sync.dma_start(dw_sb[:, 1, :, :], dw_d[:, 1, :, :])
            stem(1)
            issue_xrep(3)
            nc.sync.dma_start(dw_sb[:, 2, :, :], dw_d[:, 2, :, :])
            nc.sync.dma_start(dw_sb[:, 3, :, :], dw_d[:, 3, :, :])
            # small params (needed from phase 3 on)
            nc.sync.dma_start(lam_sb[:], lam_d[:])
            nc.sync.dma_start(bb2_sb[:], bb2_d[:])
            nc.sync.dma_start(mask_row[:], maskr_d[:][None, :])
            nc.sync.dma_start(bb2_row[:], bb2r_d[:][None, :])
            nc.sync.dma_start(downb_rep[:],
                              downbr_d[:][None, :].to_broadcast([P, D]))
            downbT = glob.tile([P, 8], f32, name="downbT")
            nc.sync.dma_start(downbT[:], downbT_d[:])
            stem(2)
            issue_xrep(4)
            down(1)
            nc.sync.dma_start(bw_sb[:], bw_d[:])
            down0_ln0()
            stem(3)
            issue_xrep(5)
            down(2)
            lnt(1)
            nc.sync.dma_start(slnw_rep[:], slnw_d[:][None, :].to_broadcast([P, D]))
            nc.sync.dma_start(ccb_rep[:], ccb_d[:][None, :].to_broadcast([P, D]))
            stem(4)
            issue_xrep(6)
            down(3)
            lnt(2)
            bproj_scan(0)
            open_group(1)
            nc.sync.dma_start(cw_sb[:], cw_d[:])
            nc.sync.dma_start(lruw_rep[:], lruw_d[:][None, :].to_broadcast([P, D]))
            nc.sync.dma_start(lrub_rep[:], lrub_d[:][None, :].to_broadcast([P, D]))
            stem(5)
            issue_xrep(7)
            down(4)
            lnt(3)
            p4(1)
            stem(6)
            issue_xrep(8)
            down(5)
            lnt(4)
            p4(2)
            stem(7)
            down(6)
            lnt(5)
            bproj_scan(1)
            open_group(2)
            stem(8)
            down(7)
            lnt(6)
            lnt(7)
            p4(3)
            p4(4)
            down(8)
            bproj_scan(2)
            open_group(3)
            lnt(8)
            bproj_scan(3)
            p4(5)
            p4(6)
            p4(7)
            p4(8)

    nc.finalize()
    return nc


def _prep_host(inputs):
    import ml_dtypes
    f = np.float32
    bf = ml_dtypes.bfloat16
    embed = np.asarray(inputs["embed"], f)
    conv_ws = [np.asarray(inputs[k], f) for k in
               ("conv1_w", "conv2_w", "conv4_w", "conv8_w")]
    conv_bs = [np.asarray(inputs[k], f) for k in
               ("conv1_b", "conv2_b", "conv4_b", "conv8_b")]
    down_w = np.asarray(inputs["down_w"], f)
    log_lam = np.asarray(inputs["log_lambda_raw"], f)
    lam = (1.0 / (1.0 + np.exp(-log_lam.astype(np.float64)))).astype(f)
    b_w = np.asarray(inputs["b_w"], f)
    c_w = np.asarray(inputs["c_w"], f)

    stem_w = np.empty((2, P, N_TAPS, 256), f)
    for kk, (ci, j, _off) in enumerate(TAPS):
        fused = embed @ conv_ws[ci][:, :, j].T        # [256v, 256c]
        stem_w[:, :, kk, :] = fused.reshape(2, P, 256)
    convb = np.concatenate(conv_bs).reshape(8, P).T.copy()      # [p, cc]

    down_wt = (down_w.transpose(1, 2, 0)                        # [d, j, e]
               .reshape(8, P, 4, D).transpose(1, 2, 0, 3).copy())  # [p, j, dc, e]
    one_m = (1.0 - lam)
    slnw = np.asarray(inputs["stem_ln_w"], f)
    slnb = np.asarray(inputs["stem_ln_b"], f)
    # values[d,t] = sum_e [(1-lam_d) b_w[d,e] slnw[e]] z^T[e,t]
    #              + (1-lam_d)(b_w[d,:] @ slnb + b_b[d])
    b_wt = ((b_w.T * one_m[None, :] * slnw[:, None])            # [e, d]
            .reshape(8, P, D).transpose(1, 0, 2).copy())        # [p, ec, d]
    bb2 = (one_m * (b_w @ slnb + np.asarray(inputs["b_b"], f))
           ).reshape(8, P).T.copy()                             # [p, dc]
    bb2_row = (one_m * (b_w @ slnb + np.asarray(inputs["b_b"], f)))  # [d]
    c_wt = c_w.T.reshape(8, P, D).transpose(1, 0, 2).copy()     # [p, dc, e]
    lam_ct = lam.reshape(8, P).T.copy()
    ccb = slnb + np.asarray(inputs["c_b"], f)

    shared = dict(
        stem_w=stem_w.astype(bf), convb=convb,
        down_wt=down_wt.astype(bf),
        downb_v=np.asarray(inputs["down_b"], f).astype(bf),
        downbT=np.asarray(inputs["down_b"], f).reshape(8, P).T.copy(),
        b_wt=b_wt.astype(bf), bb2=bb2, bb2_row=bb2_row.astype(bf),
        c_wt=c_wt.astype(bf),
        slnw_v=slnw.astype(bf), ccb_v=ccb.astype(bf),
        lruw_v=np.asarray(inputs["lru_ln_w"], f).astype(bf),
        lrub_v=np.asarray(inputs["lru_ln_b"], f).astype(bf),
        lam_ct=lam_ct,
    )

    x = np.asarray(inputs["x"]).astype(np.int64)
    in_maps = []
    for core in range(8):
        b, h = core // 2, core % 2
        t0 = h * 4096
        idx = t0 - 4 * W_SCAN - 4 + np.arange(X_LOC)
        valid = (idx >= 0) & (idx < T)
        x_loc = np.full((X_LOC,), SENTINEL, bf)
        x_loc[valid] = x[b, idx[valid]].astype(bf)
        mask = np.ones((W_MAX,), f)
        if h == 0:
            mask[:W_SCAN] = 0.0
        m = dict(shared)
        m["x_loc"] = x_loc
        m["mask_row"] = mask.astype(bf)
        in_maps.append(m)
    return in_maps


def kernel(**inputs) -> np.ndarray:
    if "nc" not in _CACHE:
        _CACHE["nc"] = _build()
    nc = _CACHE["nc"]
    in_maps = _prep_host(inputs)
    res = run_bass_kernel_spmd(nc, in_maps, list(range(8)))
    out = np.empty((B, 2048, D), np.float32)
    for core in range(8):
        b, h = core // 2, core % 2
        out[b, h * 1024:(h + 1) * 1024, :] = np.asarray(
            res.results[core]["out"], np.float32)
    return out


# revision 34
# speedup vs baseline: 1.6776x; 1.0232x over previous
"""Trainium2 Bass kernel for nn_ByteEncoder (multi-scale conv stem + per-channel LRU).

Sharding: 8 cores = (batch b in 0..3) x (time-half h in 0..1). Each core runs an
identical SPMD program over raw steps [t0-128, t0+4096) (t0 = h*4096), i.e. a
32-scan-step warmup plus its 1024 output scan steps. The warmup region is
masked to zero for h=0 cores (reference scan starts at state 0) and uses real
left-context for h=1 cores (per-channel decay lambda^32 < 1.5e-6, far below the
2e-2 tolerance).

The embedding lookup is algebraically fused into the conv stem: for one-hot
inputs, conv_k(embed[x]) == sum_taps (embed @ conv_w[:,:,j])[x[t+off]], so the
stem becomes matmuls of precontracted [256-vocab x 256-ch] tables against
one-hot columns built on-chip (iota + is_equal).

All matmuls run in bf16 (full PE rate, fast-weight-load path, half the SBUF and
HBM traffic of fp32). Everything stays in SBUF: stem and the strided down-conv
are fused per 512-step tile, and phase 3/4 (LN -> b-proj -> scan -> c-proj ->
LN) interleaves with later stem tiles so the tensor engine never idles. The
short warmup tile computes its down-conv transposed (cheap N=32 matmuls) and
normalizes across partitions via ones-matmul stats + partition_broadcast.
"""
import numpy as np

import concourse.bass as bass
import concourse.tile as tile
from concourse import mybir, bacc
from concourse.bass_utils import run_bass_kernel_spmd
from concourse.masks import make_identity

P = 128
D = 1024
B = 4
T = 8192
VOCAB = 256
SENTINEL = 512.0  # out-of-range token -> one-hot col is all zero

W_SCAN = 32             # warmup scan steps (lam^32 < 1.5e-6)
S_LOC = 1024 + W_SCAN   # scan steps computed per core
T_LOC = 4 * S_LOC       # raw steps per core (4224)
X_LOC = T_LOC + 8       # x slice incl conv halo (left 4, right 3, +1 pad)
N_CH = 8                # output chunks c1..c8 (128 scan steps each)
W_MAX = 384

f32 = mybir.dt.float32
bf16 = mybir.dt.bfloat16
AF = mybir.ActivationFunctionType
OP = mybir.AluOpType

# scan groups, in scan columns: g0 = warmup(32) + chunks 1-2, then 3/2/1 chunks.
# Last group is a single chunk so the serial tail after the last stem tile is
# as short as possible.
GROUPS = [(32 + 256), 384, 256, 128]
# chunk k (1..8) -> (group, col offset inside group)
CHUNK_POS = {1: (0, 32), 2: (0, 160), 3: (1, 0), 4: (1, 128), 5: (1, 256),
             6: (2, 0), 7: (2, 128), 8: (3, 0)}

# (conv_id, kernel_size, pad); tap offset = j - pad
CONVS = [(1, 0), (2, 1), (4, 2), (8, 4)]
TAPS = []  # (conv_id, j, off)
for ci, (K, pad) in enumerate(CONVS):
    for j in range(K):
        TAPS.append((ci, j, j - pad))
N_TAPS = len(TAPS)  # 15
TAPS_OF_CONV = [[kk for kk, (ci, _, _) in enumerate(TAPS) if ci == c] for c in range(4)]

_CACHE = {}


def _build():
    nc = bacc.Bacc()

    x_d = nc.declare_dram_parameter("x_loc", [X_LOC], bf16, isOutput=False)
    maskr_d = nc.declare_dram_parameter("mask_row", [W_MAX], bf16, isOutput=False)
    stem_d = nc.declare_dram_parameter("stem_w", [2, P, N_TAPS, 256], bf16, isOutput=False)
    convb_d = nc.declare_dram_parameter("convb", [P, 8], f32, isOutput=False)
    dw_d = nc.declare_dram_parameter("down_wt", [P, 4, 8, D], bf16, isOutput=False)
    downbr_d = nc.declare_dram_parameter("downb_v", [D], bf16, isOutput=False)
    downbT_d = nc.declare_dram_parameter("downbT", [P, 8], f32, isOutput=False)
    bw_d = nc.declare_dram_parameter("b_wt", [P, 8, D], bf16, isOutput=False)
    bb2_d = nc.declare_dram_parameter("bb2", [P, 8], f32, isOutput=False)
    bb2r_d = nc.declare_dram_parameter("bb2_row", [D], bf16, isOutput=False)
    cw_d = nc.declare_dram_parameter("c_wt", [P, 8, D], bf16, isOutput=False)
    slnw_d = nc.declare_dram_parameter("slnw_v", [D], bf16, isOutput=False)
    ccb_d = nc.declare_dram_parameter("ccb_v", [D], bf16, isOutput=False)
    lruw_d = nc.declare_dram_parameter("lruw_v", [D], bf16, isOutput=False)
    lrub_d = nc.declare_dram_parameter("lrub_v", [D], bf16, isOutput=False)
    lam_d = nc.declare_dram_parameter("lam_ct", [P, 8], f32, isOutput=False)

    out_d = nc.declare_dram_parameter("out", [1024, D], bf16, isOutput=True)

    with tile.TileContext(nc) as tc:
        with tc.tile_pool(name="glob", bufs=1) as glob, \
             tc.tile_pool(name="pw", bufs=1) as pw, \
             tc.tile_pool(name="p12t", bufs=2) as p12t, \
             tc.tile_pool(name="p34t", bufs=2) as p34t, \
             tc.tile_pool(name="ps_stem", bufs=2, space="PSUM") as ps_stem, \
             tc.tile_pool(name="ps_down", bufs=2, space="PSUM") as ps_down, \
             tc.tile_pool(name="ps_tr", bufs=1, space="PSUM") as ps_tr, \
             tc.tile_pool(name="ps_bp", bufs=1, space="PSUM") as ps_bp, \
             tc.tile_pool(name="ps_cp", bufs=2, space="PSUM") as ps_cp:

            # ---------------- on-chip constants (no DMA) -----------------
            ident = glob.tile([P, P], bf16, name="ident")
            make_identity(nc, ident)
            # ~4.5us of junk matmuls at kernel start: trips the PE HAM
            # activity window while the first weight DMAs land, so real
            # matmuls start at the full 2.4 GHz clock.
            warm_ps = ps_cp.tile([P, 512], f32, name="psc", tag="psc")
            for _ in range(34):
                nc.tensor.matmul(warm_ps[:, :128], ident[:], ident[:],
                                 start=True, stop=True)

            # ---------------- critical-path DMAs first -------------------
            stem_sb0 = pw.tile([P, N_TAPS, 256], bf16, name="stem_sb0")
            stem_sb1 = pw.tile([P, N_TAPS, 256], bf16, name="stem_sb1")
            stem_sbs = (stem_sb0, stem_sb1)
            # low-vocab halves of both tables first: the first stem matmuls
            # (cc = 0,1) touch cols 0:128 of both vocab halves
            nc.sync.dma_start(stem_sb0[:, :, 0:128], stem_d[0][:, :, 0:128])
            nc.sync.dma_start(stem_sb1[:, :, 0:128], stem_d[1][:, :, 0:128])
            convb_sb = glob.tile([P, 8], f32, name="convb_sb")
            nc.sync.dma_start(convb_sb[:], convb_d[:])

            eps_sb = glob.tile([P, 1], f32, name="eps_sb")
            nc.vector.memset(eps_sb[:], 1e-5)
            ones128 = glob.tile([P, 1], bf16, name="ones128")
            nc.vector.memset(ones128[:], 1.0)
            io0 = glob.tile([P, 1], f32, name="io0")
            io1 = glob.tile([P, 1], f32, name="io1")
            nc.gpsimd.iota(io0[:], pattern=[[0, 1]], base=0, channel_multiplier=1,
                           allow_small_or_imprecise_dtypes=True)
            nc.gpsimd.iota(io1[:], pattern=[[0, 1]], base=128, channel_multiplier=1,
                           allow_small_or_imprecise_dtypes=True)

            x_reps = {}

            def issue_xrep(tt):
                x_rep = p12t.tile([P, 520], bf16, name="x_rep", bufs=2)
                if tt == 0:
                    nc.sync.dma_start(
                        x_rep[:, :136],
                        x_d[0:136][None, :].to_broadcast([P, 136]))
                else:
                    lo = 512 * tt - 384
                    nc.sync.dma_start(
                        x_rep[:],
                        x_d[lo: lo + 520][None, :].to_broadcast([P, 520]))
                x_reps[tt] = x_rep

            issue_xrep(0)
            issue_xrep(1)
            nc.sync.dma_start(stem_sb0[:, :, 128:256], stem_d[0][:, :, 128:256])
            nc.sync.dma_start(stem_sb1[:, :, 128:256], stem_d[1][:, :, 128:256])

            # ---------------- remaining SBUF state -----------------------
            z_bf = glob.tile([P, N_CH, D], bf16, name="z_bf")
            lam_sb = glob.tile([P, 8], f32, name="lam_sb")
            bb2_sb = glob.tile([P, 8], f32, name="bb2_sb")
            mask_row = glob.tile([1, W_MAX], bf16, name="mask_row")
            bb2_row = glob.tile([1, D], bf16, name="bb2_row")
            downb_rep = glob.tile([P, D], bf16, name="downb_rep")
            dw_sb = pw.tile([P, 4, 8, D], bf16, name="dw_sb")
            bw_sb = pw.tile([P, 8, D], bf16, name="bw_sb")
            cw_sb = pw.tile([P, 8, D], bf16, name="cw_sb")
            slnw_rep = pw.tile([P, D], bf16, name="slnw_rep")
            ccb_rep = pw.tile([P, D], bf16, name="ccb_rep")
            lruw_rep = pw.tile([P, D], bf16, name="lruw_rep")
            lrub_rep = pw.tile([P, D], bf16, name="lrub_rep")

            hm_ts = {}
            hm0 = p12t.tile([P, 8, 128], bf16, name="hm0", bufs=1)

            def stem(tt):
                wid = 128 if tt == 0 else 512
                x_rep = x_reps.pop(tt)
                oh = p12t.tile([P, 2, 520], bf16, name="oh", bufs=2)
                nc.vector.tensor_scalar(out=oh[:, 0, :wid + 8],
                                        in0=x_rep[:, :wid + 8],
                                        scalar1=io0[:], scalar2=None,
                                        op0=OP.is_equal)
                nc.vector.tensor_scalar(out=oh[:, 1, :wid + 8],
                                        in0=x_rep[:, :wid + 8],
                                        scalar1=io1[:], scalar2=None,
                                        op0=OP.is_equal)
                if tt == 0:
                    hm_t = hm0
                else:
                    hm_t = p12t.tile([P, 8, 512], bf16, name="hm_t", bufs=2)
                    hm_ts[tt] = hm_t
                for cc in range(8):
                    ci, half = cc // 2, cc % 2
                    taps = TAPS_OF_CONV[ci]
                    ps = ps_stem.tile([P, 512], f32, name="pss", tag="pss")
                    n_mm = len(taps) * 2
                    i = 0
                    for vc in range(2):
                        for kk in taps:
                            off = TAPS[kk][2]
                            nc.tensor.matmul(
                                ps[:, :wid],
                                stem_sbs[vc][:, kk, half * 128:(half + 1) * 128],
                                oh[:, vc, 4 + off: 4 + off + wid],
                                start=(i == 0), stop=(i == n_mm - 1))
                            i += 1
                    nc.scalar.activation(hm_t[:, cc, :wid], ps[:, :wid],
                                         AF.Gelu, bias=convb_sb[:, cc:cc + 1])

            hd_ts = {}

            def down(c):
                """Standard down-conv for chunk c (tile c), c = 1..8."""
                hm_sb = hm_ts.pop(c)
                hd_t = p34t.tile([P, D], bf16, name="hd_t", tag="hd", bufs=2)
                hd_ts[c] = hd_t
                for eh in range(2):
                    ps = ps_down.tile([P, 512], f32, name="psd", tag="psd")
                    i = 0
                    for dc in range(8):
                        for j in range(4):
                            nc.tensor.matmul(
                                ps[:],
                                hm_sb[:, dc, j:512:4],
                                dw_sb[:, j, dc, eh * 512:(eh + 1) * 512],
                                start=(i == 0), stop=(i == 31))
                            i += 1
                    nc.vector.tensor_tensor(
                        out=hd_t[:, eh * 512:(eh + 1) * 512], in0=ps[:],
                        in1=downb_rep[:, eh * 512:(eh + 1) * 512], op=OP.add)

            def down0_ln0():
                """Warmup (32 scan steps): transposed down-conv (out [e, s],
                cheap N=32 matmuls), partition-axis LN via ones-matmul stats,
                result written straight into hsT group 0 cols 0:32."""
                hdT0 = p34t.tile([P, 8, 32], bf16, name="hdT0", bufs=1)
                for eb in range(8):
                    ps = ps_down.tile([P, 512], f32, name="psd", tag="psd")
                    i = 0
                    for dc in range(8):
                        for j in range(4):
                            nc.tensor.matmul(
                                ps[:128, :32],
                                dw_sb[:, j, dc, eb * 128:(eb + 1) * 128],
                                hm0[:, dc, j:128:4],
                                start=(i == 0), stop=(i == 31))
                            i += 1
                    # add down bias for these 128 e-channels, transposed:
                    # downb slice as per-partition scalar
                    nc.vector.tensor_scalar(
                        out=hdT0[:, eb, :], in0=ps[:128, :32],
                        scalar1=downbT[:, eb:eb + 1], scalar2=None, op0=OP.add)
                sq0 = p34t.tile([P, 8, 32], bf16, name="sq0", bufs=1)
                nc.vector.tensor_tensor(out=sq0[:], in0=hdT0[:], in1=hdT0[:],
                                        op=OP.mult)
                ps = ps_bp.tile([P, W_MAX], f32, name="psb", tag="psb")
                for eb in range(8):
                    nc.tensor.matmul(ps[0:1, 0:32], ones128[:], hdT0[:, eb, :],
                                     start=(eb == 0), stop=(eb == 7))
                for eb in range(8):
                    nc.tensor.matmul(ps[0:1, 64:96], ones128[:], sq0[:, eb, :],
                                     start=(eb == 0), stop=(eb == 7))
                m_row = p34t.tile([1, 32], f32, name="m_row", bufs=1)
                nc.vector.tensor_scalar(out=m_row[:], in0=ps[0:1, 0:32],
                                        scalar1=1.0 / D, scalar2=None,
                                        op0=OP.mult)
                v_row = p34t.tile([1, 32], f32, name="v_row", bufs=1)
                nc.vector.tensor_scalar(out=v_row[:], in0=ps[0:1, 64:96],
                                        scalar1=1.0 / D, scalar2=None,
                                        op0=OP.mult)
                msq = p34t.tile([1, 32], f32, name="msq", bufs=1)
                nc.vector.tensor_tensor(out=msq[:], in0=m_row[:], in1=m_row[:],
                                        op=OP.mult)
                nc.vector.tensor_tensor(out=v_row[:], in0=v_row[:], in1=msq[:],
                                        op=OP.subtract)
                nc.scalar.activation(v_row[:], v_row[:], AF.Sqrt,
                                     bias=eps_sb[0:1, :])
                nc.vector.reciprocal(v_row[:], v_row[:])
                # zero the warmup on h=0 cores
                nc.vector.tensor_tensor(out=v_row[:], in0=v_row[:],
                                        in1=mask_row[:, 0:32], op=OP.mult)
                m_rep = p34t.tile([P, 32], f32, name="m_rep", bufs=1)
                r_rep = p34t.tile([P, 32], f32, name="r_rep", bufs=1)
                nc.gpsimd.partition_broadcast(m_rep[:], m_row[:])
                nc.gpsimd.partition_broadcast(r_rep[:], v_row[:])
                hsT_g = hsT_tiles[0]
                for eb in range(8):
                    zt = p34t.tile([P, 32], f32, name="zt", bufs=2)
                    nc.vector.tensor_tensor(out=zt[:], in0=hdT0[:, eb, :],
                                            in1=m_rep[:], op=OP.subtract)
                    nc.vector.tensor_tensor(out=hsT_g[:, eb, 0:32], in0=zt[:],
                                            in1=r_rep[:], op=OP.mult)

            def lnt(k):
                """LN stats + z + transpose for chunk k (k = 1..8)."""
                g, off = CHUNK_POS[k]
                hd_t = hd_ts.pop(k)
                stats = p34t.tile([P, 2, 6], f32, name="stats", bufs=2)
                hd_g = hd_t[:].rearrange("p (g f) -> p g f", g=2)
                nc.vector.bn_stats(out=stats[:, 0, :], in_=hd_g[:, 0, :])
                nc.vector.bn_stats(out=stats[:, 1, :], in_=hd_g[:, 1, :])
                mv = p34t.tile([P, 2], f32, name="mv", bufs=2)
                nc.vector.bn_aggr(out=mv[:], in_=stats[:])
                rstd = p34t.tile([P, 1], f32, name="rstd", bufs=2)
                nc.scalar.activation(rstd[:], mv[:, 1:2], AF.Sqrt, bias=eps_sb[:])
                nc.vector.reciprocal(rstd[:], rstd[:])
                nc.vector.tensor_scalar(out=z_bf[:, k - 1, :], in0=hd_t[:],
                                        scalar1=mv[:, 0:1], scalar2=rstd[:],
                                        op0=OP.subtract, op1=OP.mult)
                pst = ps_tr.tile([P, 8, P], bf16, name="pst", tag="pst")
                for ec in range(8):
                    nc.tensor.transpose(
                        pst[:, ec, :], z_bf[:, k - 1, ec * 128:(ec + 1) * 128],
                        ident[:])
                hsT_g = hsT_tiles[g]
                nc.scalar.copy(hsT_g[:, :, off:off + 128], pst[:])

            hsT_tiles = {}
            h_tiles = {}

            def open_group(g):
                hsT_tiles[g] = p34t.tile([P, 8, W_MAX], bf16, name="hsT",
                                         tag="hsT", bufs=2)

            def bproj_scan(g):
                W = GROUPS[g]
                hsT_g = hsT_tiles[g]
                h_g = p34t.tile([P, 8, W_MAX], bf16, name="h_g", tag="h_g",
                                bufs=2)
                h_tiles[g] = h_g
                for dc in range(8):
                    psb = ps_bp.tile([P, W_MAX], f32, name="psb", tag="psb")
                    if g == 0:
                        # masked per-channel bias via 1-row matmul
                        nc.tensor.matmul(psb[:, :W],
                                         bb2_row[:, dc * 128:(dc + 1) * 128],
                                         mask_row[:, :W],
                                         start=True, stop=False)
                    for ec in range(8):
                        nc.tensor.matmul(
                            psb[:, :W],
                            bw_sb[:, ec, dc * 128:(dc + 1) * 128],
                            hsT_g[:, ec, :W],
                            start=(g != 0 and ec == 0), stop=(ec == 7))
                    vals = p34t.tile([P, W_MAX], bf16, name="vals", bufs=2)
                    if g == 0:
                        nc.vector.tensor_scalar(out=vals[:, :W],
                                                in0=psb[:, :W], scalar1=0.0,
                                                scalar2=None, op0=OP.add)
                    else:
                        nc.vector.tensor_scalar(out=vals[:, :W],
                                                in0=psb[:, :W],
                                                scalar1=bb2_sb[:, dc:dc + 1],
                                                scalar2=None, op0=OP.add)
                    init = (0.0 if g == 0
                            else h_tiles[g - 1][:, dc,
                                                GROUPS[g - 1] - 1: GROUPS[g - 1]])
                    nc.vector.tensor_tensor_scan(
                        out=h_g[:, dc, :W],
                        data0=lam_sb[:, dc:dc + 1].to_broadcast([P, W]),
                        data1=vals[:, :W],
                        initial=init, op0=OP.mult, op1=OP.add)

            def p4(k):
                """c-proj + residual + final LN -> out rows (k-1)*128.."""
                g, off = CHUNK_POS[k]
                h_g = h_tiles[g]
                res_b = p34t.tile([P, D], bf16, name="res_b", bufs=2)
                nc.gpsimd.tensor_tensor(out=res_b[:], in0=z_bf[:, k - 1, :],
                                        in1=slnw_rep[:], op=OP.mult)
                nc.gpsimd.tensor_tensor(out=res_b[:], in0=res_b[:],
                                        in1=ccb_rep[:], op=OP.add)
                res_f = p34t.tile([P, D], bf16, name="res_f", bufs=2)
                for eh in range(2):
                    psc = ps_cp.tile([P, 512], f32, name="psc", tag="psc")
                    for dc in range(8):
                        nc.tensor.matmul(
                            psc[:],
                            h_g[:, dc, off:off + 128],
                            cw_sb[:, dc, eh * 512:(eh + 1) * 512],
                            start=(dc == 0), stop=(dc == 7))
                    nc.vector.tensor_tensor(
                        out=res_f[:, eh * 512:(eh + 1) * 512], in0=psc[:],
                        in1=res_b[:, eh * 512:(eh + 1) * 512], op=OP.add)
                stats2 = p34t.tile([P, 2, 6], f32, name="stats2", bufs=2)
                res_g = res_f[:].rearrange("p (g f) -> p g f", g=2)
                nc.vector.bn_stats(out=stats2[:, 0, :], in_=res_g[:, 0, :])
                nc.vector.bn_stats(out=stats2[:, 1, :], in_=res_g[:, 1, :])
                mv2 = p34t.tile([P, 2], f32, name="mv2", bufs=2)
                nc.vector.bn_aggr(out=mv2[:], in_=stats2[:])
                rstd2 = p34t.tile([P, 1], f32, name="rstd2", bufs=2)
                nc.scalar.activation(rstd2[:], mv2[:, 1:2], AF.Sqrt,
                                     bias=eps_sb[:])
                nc.vector.reciprocal(rstd2[:], rstd2[:])
                nc.vector.tensor_scalar(out=res_f[:], in0=res_f[:],
                                        scalar1=mv2[:, 0:1], scalar2=rstd2[:],
                                        op0=OP.subtract, op1=OP.mult)
                nc.vector.tensor_tensor(out=res_f[:], in0=res_f[:],
                                        in1=lruw_rep[:], op=OP.mult)
                nc.vector.tensor_tensor(out=res_f[:], in0=res_f[:],
                                        in1=lrub_rep[:], op=OP.add)
                nc.sync.dma_start(out_d[(k - 1) * 128: k * 128, :], res_f[:])

            # ---------------- software-pipelined emission ----------------
            open_group(0)
            stem(0)
            issue_xrep(2)
            nc.sync.dma_start(dw_sb[:, 0, :, :], dw_d[:, 0, :, :])
            nc.sync.dma_start(dw_sb[:, 1, :, :], dw_d[:, 1, :, :])
            stem(1)
            issue_xrep(3)
            nc.sync.dma_start(dw_sb[:, 2, :, :], dw_d[:, 2, :, :])
            nc.sync.dma_start(dw_sb[:, 3, :, :], dw_d[:, 3, :, :])
            # small params (needed from phase 3 on)
            nc.sync.dma_start(lam_sb[:], lam_d[:])
            nc.sync.dma_start(bb2_sb[:], bb2_d[:])
            nc.sync.dma_start(mask_row[:], maskr_d[:][None, :])
            nc.sync.dma_start(bb2_row[:], bb2r_d[:][None, :])
            nc.sync.dma_start(downb_rep[:],
                              downbr_d[:][None, :].to_broadcast([P, D]))
            downbT = glob.tile([P, 8], f32, name="downbT")
            nc.sync.dma_start(downbT[:], downbT_d[:])
            stem(2)
            issue_xrep(4)
            down(1)
            nc.sync.dma_start(bw_sb[:], bw_d[:])
            down0_ln0()
            stem(3)
            issue_xrep(5)
            down(2)
            lnt(1)
            nc.sync.dma_start(slnw_rep[:], slnw_d[:][None, :].to_broadcast([P, D]))
            nc.sync.dma_start(ccb_rep[:], ccb_d[:][None, :].to_broadcast([P, D]))
            stem(4)
            issue_xrep(6)
            down(3)
            lnt(2)
            bproj_scan(0)
            open_group(1)
            nc.sync.dma_start(cw_sb[:], cw_d[:])
            nc.sync.dma_start(lruw_rep[:], lruw_d[:][None, :].to_broadcast([P, D]))
            nc.sync.dma_start(lrub_rep[:], lrub_d[:][None, :].to_broadcast([P, D]))
            stem(5)
            issue_xrep(7)
            down(4)
            lnt(3)
            p4(1)
            stem(6)
            issue_xrep(8)
            down(5)
            lnt(4)
            p4(2)
            stem(7)
            down(6)
            lnt(5)
            bproj_scan(1)
            open_group(2)
            stem(8)
            down(7)
            lnt(6)
            lnt(7)
            p4(3)
            p4(4)
            down(8)
            bproj_scan(2)
            open_group(3)
            lnt(8)
            bproj_scan(3)
            p4(5)
            p4(6)
            p4(7)
            p4(8)

    nc.finalize()
    return nc


def _prep_host(inputs):
    import ml_dtypes
    f = np.float32
    bf = ml_dtypes.bfloat16
    embed = np.asarray(inputs["embed"], f)
    conv_ws = [np.asarray(inputs[k], f) for k in
               ("conv1_w", "conv2_w", "conv4_w", "conv8_w")]
    conv_bs = [np.asarray(inputs[k], f) for k in
               ("conv1_b", "conv2_b", "conv4_b", "conv8_b")]
    down_w = np.asarray(inputs["down_w"], f)
    log_lam = np.asarray(inputs["log_lambda_raw"], f)
    lam = (1.0 / (1.0 + np.exp(-log_lam.astype(np.float64)))).astype(f)
    b_w = np.asarray(inputs["b_w"], f)
    c_w = np.asarray(inputs["c_w"], f)

    stem_w = np.empty((2, P, N_TAPS, 256), f)
    for kk, (ci, j, _off) in enumerate(TAPS):
        fused = embed @ conv_ws[ci][:, :, j].T        # [256v, 256c]
        stem_w[:, :, kk, :] = fused.reshape(2, P, 256)
    convb = np.concatenate(conv_bs).reshape(8, P).T.copy()      # [p, cc]

    down_wt = (down_w.transpose(1, 2, 0)                        # [d, j, e]
               .reshape(8, P, 4, D).transpose(1, 2, 0, 3).copy())  # [p, j, dc, e]
    one_m = (1.0 - lam)
    slnw = np.asarray(inputs["stem_ln_w"], f)
    slnb = np.asarray(inputs["stem_ln_b"], f)
    # values[d,t] = sum_e [(1-lam_d) b_w[d,e] slnw[e]] z^T[e,t]
    #              + (1-lam_d)(b_w[d,:] @ slnb + b_b[d])
    b_wt = ((b_w.T * one_m[None, :] * slnw[:, None])            # [e, d]
            .reshape(8, P, D).transpose(1, 0, 2).copy())        # [p, ec, d]
    bb2 = (one_m * (b_w @ slnb + np.asarray(inputs["b_b"], f))
           ).reshape(8, P).T.copy()                             # [p, dc]
    bb2_row = (one_m * (b_w @ slnb + np.asarray(inputs["b_b"], f)))  # [d]
    c_wt = c_w.T.reshape(8, P, D).transpose(1, 0, 2).copy()     # [p, dc, e]
    lam_ct = lam.reshape(8, P).T.copy()
    ccb = slnb + np.asarray(inputs["c_b"], f)

    shared = dict(
        stem_w=stem_w.astype(bf), convb=convb,
        down_wt=down_wt.astype(bf),
        downb_v=np.asarray(inputs["down_b"], f).astype(bf),
        downbT=np.asarray(inputs["down_b"], f).reshape(8, P).T.copy(),
        b_wt=b_wt.astype(bf), bb2=bb2, bb2_row=bb2_row.astype(bf),
        c_wt=c_wt.astype(bf),
        slnw_v=slnw.astype(bf), ccb_v=ccb.astype(bf),
        lruw_v=np.asarray(inputs["lru_ln_w"], f).astype(bf),
        lrub_v=np.asarray(inputs["lru_ln_b"], f).astype(bf),
        lam_ct=lam_ct,
    )

    x = np.asarray(inputs["x"]).astype(np.int64)
    in_maps = []
    for core in range(8):
        b, h = core // 2, core % 2
        t0 = h * 4096
        idx = t0 - 4 * W_SCAN - 4 + np.arange(X_LOC)
        valid = (idx >= 0) & (idx < T)
        x_loc = np.full((X_LOC,), SENTINEL, bf)
        x_loc[valid] = x[b, idx[valid]].astype(bf)
        mask = np.ones((W_MAX,), f)
        if h == 0:
            mask[:W_SCAN] = 0.0
        m = dict(shared)
        m["x_loc"] = x_loc
        m["mask_row"] = mask.astype(bf)
        in_maps.append(m)
    return in_maps


def kernel(**inputs) -> np.ndarray:
    if "nc" not in _CACHE:
        _CACHE["nc"] = _build()
    nc = _CACHE["nc"]
    in_maps = _prep_host(inputs)
    res = run_bass_kernel_spmd(nc, in_maps, list(range(8)))
    out = np.empty((B, 2048, D), np.float32)
    for core in range(8):
        b, h = core // 2, core % 2
        out[b, h * 1024:(h + 1) * 1024, :] = np.asarray(
            res.results[core]["out"], np.float32)
    return out
